# revision 18
# baseline (speedup 1.0000x reference)
"""BitNet attention TRN2 kernel: 8-way SPMD (2 heads/core, tokens sharded 8-way).

Dataflow per core c (tokens Tc = rows [c*L,(c+1)*L) of the flattened [B*T, D]
activations, heads {2c, 2c+1}):
  A) x_had = x @ H  via fp16 hi/lo split matmuls (fp32 accumulate);
     per-token int8 absmax quant -> y (fp16-held small ints); PE-transpose;
     AllGather y.T and the per-token absmax across the 8 cores.
  B) Q/K/V projections in the integer domain (exact in fp16), dequantized with
     per-token scales (DMA-broadcast absmax vector x host weight-scale consts).
  C) Per (batch, head): S.T = Ks.T^T @ Qs.T (row-packed head pairs), exp on ACT,
     out.T = [V | 1]^T @ expS.T accumulated over key tiles (ones column gives the
     softmax denominator), normalize, PE-transpose to token-major, AllToAll so
     each core gets its own tokens x all 1024 columns.
  D) Second absmax quant, z = y2 @ Wo_u.T (integer domain), per-token dequant.

Host side quantizes weights to ternary {-1,0,1} (fp16-exact), splits/transposes
x, and concatenates the 8 z slices.
"""
import sys

if "/opt/trn_rl_repo" not in sys.path:
    sys.path.insert(0, "/opt/trn_rl_repo")

import numpy as np

P = 128
D = 1024
NH = 16
DH = 64
B = 2
N_CORES = 8
MAGIC = 12582912.0  # 1.5 * 2**23: fp32 round-to-nearest-int via add/sub

_BUILD_CACHE = {}


def _build(T):
    import concourse.bass as bass  # noqa: F401
    import concourse.mybir as mybir
    import concourse.tile as tile
    from concourse import bacc
    from concourse.masks import make_identity

    f16 = mybir.dt.float16
    f32 = mybir.dt.float32
    i8 = mybir.dt.int8
    Exp = mybir.ActivationFunctionType.Exp
    mult = mybir.AluOpType.mult
    add = mybir.AluOpType.add
    subtract = mybir.AluOpType.subtract
    X = mybir.AxisListType.X

    BT = B * T
    L = BT // N_CORES          # tokens per core
    NT = L // P                # local token tiles
    DK = D // P                # contraction chunks
    QB = 512                   # query block
    NQB = T // QB              # query blocks per batch
    NKT = T // P               # key tiles per batch
    VT = BT // P               # global token tiles (for V)
    GROUPS = [list(range(N_CORES))]

    nc = bacc.Bacc("TRN2", target_bir_lowering=False, debug=False,
                   num_devices=N_CORES)

    # I/O
    xT_hi = nc.dram_tensor("xT_hi", [D, L], f16, kind="ExternalInput")
    xT_lo = nc.dram_tensor("xT_lo", [D, L], f16, kind="ExternalInput")
    Hm = nc.dram_tensor("Hm", [D, D], f16, kind="ExternalInput")
    WqT = nc.dram_tensor("WqT", [D, P], f16, kind="ExternalInput")
    WkT = nc.dram_tensor("WkT", [D, P], f16, kind="ExternalInput")
    WvT = nc.dram_tensor("WvT", [D, P], f16, kind="ExternalInput")
    WoT = nc.dram_tensor("WoT", [D, D], f16, kind="ExternalInput")
    consts = nc.dram_tensor("consts", [1, 4], f32, kind="ExternalInput")
    z = nc.dram_tensor("z", [L, D], f32, kind="ExternalOutput")

    with tile.TileContext(nc) as tc:
        cpool = tc.alloc_tile_pool(name="cpool", bufs=1)
        dram = tc.alloc_tile_pool(name="dram", bufs=1, space="DRAM")

        ident16 = cpool.tile([P, P], f16)
        make_identity(nc, ident16)
        ident32 = cpool.tile([P, P], f32)
        make_identity(nc, ident32)
        csb = cpool.tile([P, 4], f32)
        nc.sync.dma_start(csb, consts.ap().to_broadcast((P, 4)))

        # DRAM intermediates
        yT_loc = dram.tile([D + 4, L], i8)
        yT_g = dram.tile([N_CORES * (D + 4), L], i8, addr_space="Shared")
        a2a_in = dram.tile([N_CORES * P, L], f32)
        a2a_out = dram.tile([N_CORES * P, L], f32)
        y2_loc = dram.tile([L, D], f16)

        # ---------------- Phase A: x@H, quant, transpose, gather ----------
        with tc.tile_pool(name="pre", bufs=1) as pre, \
             tc.tile_pool(name="workA", bufs=3) as workA, \
             tc.tile_pool(name="psA", bufs=2, space="PSUM") as psA, \
             tc.tile_pool(name="psT", bufs=3, space="PSUM") as psT:
            sA = nc.named_scope("phaseA"); sA.__enter__()
            xhi = pre.tile([P, DK, L], f16)
            xlo = pre.tile([P, DK, L], f16)
            Hsb = pre.tile([P, DK, D], f16)
            xhi_v = xT_hi.ap().rearrange("(o p) t -> p o t", p=P)
            xlo_v = xT_lo.ap().rearrange("(o p) t -> p o t", p=P)
            H_v = Hm.ap().rearrange("(o p) d -> p o d", p=P)
            for kc in range(DK):
                nc.sync.dma_start(Hsb[:, kc], H_v[:, kc])
                nc.sync.dma_start(xhi[:, kc], xhi_v[:, kc])
                nc.sync.dma_start(xlo[:, kc], xlo_v[:, kc])
            yT_sb = pre.tile([P, DK, L], i8)
            am_all = pre.tile([P, NT], f32)

            for tt in range(NT):
                ps = psA.tile([P, 1024], f32, tag="xh")
                for half in range(2):
                    for kc in range(DK):
                        nc.tensor.matmul(
                            ps[:, half * 512:(half + 1) * 512],
                            xhi[:, kc, tt * P:(tt + 1) * P],
                            Hsb[:, kc, half * 512:(half + 1) * 512],
                            start=(kc == 0), stop=False)
                        nc.tensor.matmul(
                            ps[:, half * 512:(half + 1) * 512],
                            xlo[:, kc, tt * P:(tt + 1) * P],
                            Hsb[:, kc, half * 512:(half + 1) * 512],
                            start=False, stop=(kc == DK - 1))
                am_t = am_all[:, tt:tt + 1]
                nc.vector.reduce_max(am_t, ps, axis=X, apply_absolute_value=True)
                nc.vector.tensor_scalar_max(am_t, am_t, 1e-5)
                rec = workA.tile([P, 1], f32, tag="rec")
                nc.vector.reciprocal(rec, am_t)
                s127 = workA.tile([P, 1], f32, tag="s127")
                nc.vector.tensor_scalar_mul(s127, rec, 127.0)
                tmp = workA.tile([P, 1024], f32, tag="tmpA")
                nc.vector.tensor_scalar(tmp, ps, s127, MAGIC, mult, add)
                y_t = workA.tile([P, 1024], f16, tag="yA")
                nc.vector.tensor_scalar(y_t, tmp, MAGIC, None, subtract)
                for kc in range(DK):
                    pst = psT.tile([P, P], f16, tag="trA")
                    nc.tensor.transpose(pst, y_t[:, kc * P:(kc + 1) * P], ident16)
                    nc.vector.tensor_copy(yT_sb[:, kc, tt * P:(tt + 1) * P], pst)
            sA.__exit__(None, None, None)
            sG = nc.named_scope("gather"); sG.__enter__()
            nc.sync.dma_start(yT_loc[0:D, :].rearrange("(o p) t -> p o t", p=P),
                              yT_sb)
            am_bytes = yT_loc.rearrange("a b -> (a b)")[D * L:(D + 4) * L]
            nc.sync.dma_start(
                am_bytes.bitcast(f32).rearrange("(t p) -> p t", p=P), am_all)
            nc.gpsimd.collective_compute(
                "AllGather", mybir.AluOpType.bypass, replica_groups=GROUPS,
                ins=[yT_loc.opt()], outs=[yT_g.opt()])
            sG.__exit__(None, None, None)

        # ---------------- Phase B: QKV ------------------------------------
        fin = tc.alloc_tile_pool(name="fin", bufs=1)
        attn = tc.alloc_tile_pool(name="attn", bufs=1)
        QsT = attn.tile([P, BT], f16)
        KsT = attn.tile([P, BT], f16)
        V_A = attn.tile([P, VT, 65], f16)
        V_B = attn.tile([P, VT, 65], f16)

        with tc.tile_pool(name="gath", bufs=1) as gath, \
             tc.tile_pool(name="workB", bufs=3) as workB, \
             tc.tile_pool(name="psB", bufs=2, space="PSUM") as psB:
            sB = nc.named_scope("phaseB"); sB.__enter__()
            yTg = gath.tile([P, DK, BT], f16)
            yt_flat = yT_g.rearrange("a b -> (a b)")
            for peer in range(N_CORES):
                stg = workB.tile([P, DK, L], i8, tag="stg")
                blk = yt_flat[peer * (D + 4) * L:peer * (D + 4) * L + D * L]
                nc.sync.dma_start(
                    stg, blk.rearrange("(o p t) -> p o t", p=P, t=L))
                nc.vector.tensor_copy(yTg[:, :, peer * L:(peer + 1) * L], stg)
            amf = [yt_flat[a * (D + 4) * L + D * L:a * (D + 4) * L + (D + 4) * L]
                   .bitcast(f32) for a in range(N_CORES)]

            A_q = gath.tile([P, BT], f32)
            A_k = gath.tile([P, BT], f32)
            for a in range(N_CORES):
                nc.sync.dma_start(
                    A_k[:, a * L:(a + 1) * L],
                    amf[a][None, :].to_broadcast((P, L)))
            nc.vector.tensor_scalar(A_q, A_k, csb[:, 0:1], None, mult)
            nc.vector.tensor_scalar(A_k, A_k, 1.0 / 127.0, None, mult)
            Av = gath.tile([P, VT], f32)
            for a in range(N_CORES):
                nc.sync.dma_start(
                    Av[:, a * NT:(a + 1) * NT],
                    amf[a].rearrange("(t p) -> p t", p=P))
            nc.vector.tensor_scalar(Av, Av, csb[:, 1:2], None, mult)

            wq = gath.tile([P, DK, P], f16)
            nc.sync.dma_start(wq, WqT.ap().rearrange("(o p) m -> p o m", p=P))
            wk = gath.tile([P, DK, P], f16)
            nc.sync.dma_start(wk, WkT.ap().rearrange("(o p) m -> p o m", p=P))
            wv = gath.tile([P, DK, P], f16)
            nc.sync.dma_start(wv, WvT.ap().rearrange("(o p) m -> p o m", p=P))

            TBW = min(512, L // 2)
            for tb in range(BT // TBW):
                sl = slice(tb * TBW, (tb + 1) * TBW)
                psq = psB.tile([P, TBW], f32, tag="psq")
                for kc in range(DK):
                    nc.tensor.matmul(psq, wq[:, kc], yTg[:, kc, sl],
                                     start=(kc == 0), stop=(kc == DK - 1))
                nc.vector.tensor_tensor(QsT[:, sl], psq, A_q[:, sl], mult)
                psk = psB.tile([P, TBW], f32, tag="psk")
                for kc in range(DK):
                    nc.tensor.matmul(psk, wk[:, kc], yTg[:, kc, sl],
                                     start=(kc == 0), stop=(kc == DK - 1))
                nc.vector.tensor_tensor(KsT[:, sl], psk, A_k[:, sl], mult)

            nc.vector.memset(V_A[:, :, 64:65], 1.0)
            nc.vector.memset(V_B[:, :, 64:65], 1.0)
            for vt in range(VT):
                psv = psB.tile([P, P], f32, tag="psv")
                for kc in range(DK):
                    nc.tensor.matmul(psv, yTg[:, kc, vt * P:(vt + 1) * P],
                                     wv[:, kc],
                                     start=(kc == 0), stop=(kc == DK - 1))
                nc.vector.tensor_scalar(V_A[:, vt, 0:64], psv[:, 0:64],
                                        Av[:, vt:vt + 1], None, mult)
                nc.vector.tensor_scalar(V_B[:, vt, 0:64], psv[:, 64:128],
                                        Av[:, vt:vt + 1], None, mult)

        # ---------------- Phase C: attention ------------------------------
        sB.__exit__(None, None, None)
        wo = fin.tile([P, DK, D], f16)
        nc.sync.dma_start(wo, WoT.ap().rearrange("(o p) n -> p o n", p=P))

        with tc.tile_pool(name="workC", bufs=6) as workC, \
             tc.tile_pool(name="dramC", bufs=3, space="DRAM") as dramC, \
             tc.tile_pool(name="psS", bufs=2, space="PSUM") as psS_pool, \
             tc.tile_pool(name="psO", bufs=2, space="PSUM") as psO_pool:
            sC = nc.named_scope("phaseC"); sC.__enter__()
            a2a_in_v = a2a_in.rearrange("(a p) l -> a p l", a=N_CORES)
            for b in range(B):
                for qb in range(NQB):
                    q0 = b * T + qb * QB
                    poA = psO_pool.tile([P, 512], f32, tag="poA")
                    poB = psO_pool.tile([P, 512], f32, tag="poB")
                    for kt in range(NKT):
                        k0 = b * T + kt * P
                        ps = psS_pool.tile([P, 1024], f32, tag="S")
                        nc.tensor.matmul(ps[:, 0:512],
                                         KsT[0:64, k0:k0 + P],
                                         QsT[0:64, q0:q0 + QB],
                                         start=True, stop=True)
                        nc.tensor.matmul(ps[:, 512:1024],
                                         KsT[64:128, k0:k0 + P],
                                         QsT[64:128, q0:q0 + QB],
                                         start=True, stop=True)
                        ex = workC.tile([P, 1024], f16, tag="ex")
                        nc.scalar.activation(ex, ps, Exp)
                        vt = (b * T) // P + kt
                        nc.tensor.matmul(poA[0:65], V_A[:, vt], ex[:, 0:512],
                                         start=(kt == 0), stop=(kt == NKT - 1))
                        nc.tensor.matmul(poB[0:65], V_B[:, vt], ex[:, 512:1024],
                                         start=(kt == 0), stop=(kt == NKT - 1))
                    for head, po in ((0, poA), (1, poB)):
                        rrow = workC.tile([1, QB], f32, tag="rrow")
                        nc.vector.reciprocal(rrow, po[64:65, 0:QB])
                        rdr = dramC.tile([1, QB], f32, tag="rdr")
                        nc.sync.dma_start(rdr, rrow)
                        rbc = workC.tile([64, QB], f32, tag="rbc")
                        nc.sync.dma_start(rbc, rdr.to_broadcast((64, QB)))
                        onrm = workC.tile([64, QB], f32, tag="onrm")
                        nc.vector.tensor_tensor(onrm, po[0:64], rbc, mult)
                        step = min(QB, L)
                        for j in range(QB // step):
                            peer, tl = divmod(q0 + j * step, L)
                            nc.sync.dma_start(
                                a2a_in_v[peer, head * 64:(head + 1) * 64,
                                         tl:tl + step],
                                onrm[:, j * step:(j + 1) * step])

            sC.__exit__(None, None, None)
            sA2A = nc.named_scope("a2a"); sA2A.__enter__()
            nc.gpsimd.collective_compute(
                "AllToAll", mybir.AluOpType.bypass, replica_groups=GROUPS,
                ins=[a2a_in.opt()], outs=[a2a_out.opt()])
            sA2A.__exit__(None, None, None)

        attn.release()

        # ---------------- Phase D: final quant + output projection --------
        with tc.tile_pool(name="workD", bufs=3) as workD, \
             tc.tile_pool(name="finD", bufs=1) as finD, \
             tc.tile_pool(name="psD", bufs=2, space="PSUM") as psD, \
             tc.tile_pool(name="psTD", bufs=3, space="PSUM") as psTD:
            sD = nc.named_scope("phaseD"); sD.__enter__()
            outT = finD.tile([P, DK, L], f32)
            a2a_out_v = a2a_out.rearrange("(a p) l -> a p l", a=N_CORES)
            for peer in range(N_CORES):
                nc.sync.dma_start(outT[:, peer, :], a2a_out_v[peer])
            outf = finD.tile([P, NT, D], f32)
            for tt in range(NT):
                for o in range(DK):
                    pst = psTD.tile([P, P], f32, tag="trD2")
                    nc.tensor.transpose(pst, outT[:, o, tt * P:(tt + 1) * P],
                                        ident32)
                    nc.vector.tensor_copy(outf[:, tt, o * P:(o + 1) * P], pst)
            y2T = finD.tile([P, DK, L], f16)
            a2r = finD.tile([P, NT], f32)
            for tt in range(NT):
                am2 = workD.tile([P, 1], f32, tag="am2")
                nc.vector.reduce_max(am2, outf[:, tt], axis=X,
                                     apply_absolute_value=True)
                nc.vector.tensor_scalar_max(am2, am2, 1e-5)
                nc.vector.tensor_tensor(a2r[:, tt:tt + 1], am2, csb[:, 2:3], mult)
                rec = workD.tile([P, 1], f32, tag="recD")
                nc.vector.reciprocal(rec, am2)
                s127 = workD.tile([P, 1], f32, tag="s127D")
                nc.vector.tensor_scalar_mul(s127, rec, 127.0)
                tmp = workD.tile([P, D], f32, tag="tmpD")
                nc.vector.tensor_scalar(tmp, outf[:, tt], s127, MAGIC, mult, add)
                y2 = workD.tile([P, D], f16, tag="y2")
                nc.vector.tensor_scalar(y2, tmp, MAGIC, None, subtract)
                for kc in range(DK):
                    pst = psTD.tile([P, P], f16, tag="trD")
                    nc.tensor.transpose(pst, y2[:, kc * P:(kc + 1) * P], ident16)
                    nc.vector.tensor_copy(y2T[:, kc, tt * P:(tt + 1) * P], pst)
            for tt in range(NT):
                for nh in range(2):
                    psz = psD.tile([P, 512], f32, tag="psz")
                    for kc in range(DK):
                        nc.tensor.matmul(psz, y2T[:, kc, tt * P:(tt + 1) * P],
                                         wo[:, kc, nh * 512:(nh + 1) * 512],
                                         start=(kc == 0), stop=(kc == DK - 1))
                    zsb = workD.tile([P, 512], f32, tag="zsb")
                    nc.vector.tensor_scalar(zsb, psz, a2r[:, tt:tt + 1], None,
                                            mult)
                    nc.sync.dma_start(
                        z.ap()[tt * P:(tt + 1) * P, nh * 512:(nh + 1) * 512],
                        zsb)

        sD.__exit__(None, None, None)
        fin.release()
        dram.release()
        cpool.release()

    nc.compile()
    return nc


def _get_nc(T):
    if T not in _BUILD_CACHE:
        _BUILD_CACHE[T] = _build(T)
    return _BUILD_CACHE[T]


def _wquant(w):
    # reference: scale = 1/clip(mean|w|,1e-5); u = clip(round(w*scale),-1,1)/scale
    scale = np.float32(1.0) / np.maximum(
        np.float32(np.mean(np.abs(w), dtype=np.float64)), np.float32(1e-5))
    u = np.clip(np.rint(w * scale), -1, 1).astype(np.float32)
    return u, np.float32(1.0) / scale  # ternary, dequant scale (= clipped mean)


def kernel(x, mask, Wq, Wk, Wv, Wo, H):
    from concourse.bass_utils import run_bass_kernel_spmd

    x = np.asarray(x, np.float32)
    Wq = np.asarray(Wq, np.float32); Wk = np.asarray(Wk, np.float32)
    Wv = np.asarray(Wv, np.float32); Wo = np.asarray(Wo, np.float32)
    H = np.asarray(H, np.float32)
    Bx, T, Dx = x.shape
    BT = Bx * T
    L = BT // N_CORES

    nc = _get_nc(T)

    xf = x.reshape(BT, Dx)
    x_hi = xf.astype(np.float16)
    x_lo = (xf - x_hi.astype(np.float32)).astype(np.float16)
    H16 = H.astype(np.float16)

    uq, cq = _wquant(Wq); uk, ck = _wquant(Wk)
    uv, cv = _wquant(Wv); uo, co = _wquant(Wo)
    uqT = np.ascontiguousarray(uq.T.astype(np.float16))
    ukT = np.ascontiguousarray(uk.T.astype(np.float16))
    uvT = np.ascontiguousarray(uv.T.astype(np.float16))
    uoT = np.ascontiguousarray(uo.T.astype(np.float16))

    c0 = np.float32(cq) * np.float32(ck) / (np.float32(np.sqrt(DH)) * np.float32(127.0))
    c1 = np.float32(cv) / np.float32(127.0)
    c2 = np.float32(co) / np.float32(127.0)
    consts = np.array([[c0, c1, c2, 0.0]], np.float32)

    in_maps = []
    for c in range(N_CORES):
        rows = slice(c * L, (c + 1) * L)
        cols = slice(c * P, (c + 1) * P)
        in_maps.append({
            "xT_hi": np.ascontiguousarray(x_hi[rows].T),
            "xT_lo": np.ascontiguousarray(x_lo[rows].T),
            "Hm": H16,
            "WqT": np.ascontiguousarray(uqT[:, cols]),
            "WkT": np.ascontiguousarray(ukT[:, cols]),
            "WvT": np.ascontiguousarray(uvT[:, cols]),
            "WoT": uoT,
            "consts": consts,
        })

    res = run_bass_kernel_spmd(nc, in_maps, core_ids=list(range(N_CORES)))
    kernel.last_results = res
    z = np.concatenate([res.results[c]["z"] for c in range(N_CORES)], axis=0)
    return z.reshape(Bx, T, Dx).astype(np.float32)


# revision 19
# speedup vs baseline: 1.0459x; 1.0459x over previous
"""BitNet attention TRN2 kernel: 8-way SPMD (2 heads/core, tokens sharded 8-way).

Dataflow per core c (tokens Tc = rows [c*L,(c+1)*L) of the flattened [B*T, D]
activations, heads {2c, 2c+1}):
  A) x_had = x @ H  via fp16 hi/lo split matmuls (fp32 accumulate);
     per-token int8 absmax quant -> y (fp16-held small ints); PE-transpose;
     AllGather y.T and the per-token absmax across the 8 cores.
  B) Q/K/V projections in the integer domain (exact in fp16), dequantized with
     per-token scales (DMA-broadcast absmax vector x host weight-scale consts).
  C) Per (batch, head): S.T = Ks.T^T @ Qs.T (row-packed head pairs), exp on ACT,
     out.T = [V | 1]^T @ expS.T accumulated over key tiles (ones column gives the
     softmax denominator), normalize, PE-transpose to token-major, AllToAll so
     each core gets its own tokens x all 1024 columns.
  D) Second absmax quant, z = y2 @ Wo_u.T (integer domain), per-token dequant.

Host side quantizes weights to ternary {-1,0,1} (fp16-exact), splits/transposes
x, and concatenates the 8 z slices.
"""
import sys

if "/opt/trn_rl_repo" not in sys.path:
    sys.path.insert(0, "/opt/trn_rl_repo")

import numpy as np

P = 128
D = 1024
NH = 16
DH = 64
B = 2
N_CORES = 8
MAGIC = 12582912.0  # 1.5 * 2**23: fp32 round-to-nearest-int via add/sub

_BUILD_CACHE = {}


def _build(T):
    import concourse.bass as bass  # noqa: F401
    import concourse.mybir as mybir
    import concourse.tile as tile
    from concourse import bacc
    from concourse.masks import make_identity

    f16 = mybir.dt.float16
    f32 = mybir.dt.float32
    i8 = mybir.dt.int8
    Exp = mybir.ActivationFunctionType.Exp
    mult = mybir.AluOpType.mult
    add = mybir.AluOpType.add
    subtract = mybir.AluOpType.subtract
    X = mybir.AxisListType.X

    BT = B * T
    L = BT // N_CORES          # tokens per core
    NT = L // P                # local token tiles
    DK = D // P                # contraction chunks
    QB = 512                   # query block
    NQB = T // QB              # query blocks per batch
    NKT = T // P               # key tiles per batch
    VT = BT // P               # global token tiles (for V)
    GROUPS = [list(range(N_CORES))]

    nc = bacc.Bacc("TRN2", target_bir_lowering=False, debug=False,
                   num_devices=N_CORES)

    # I/O
    xT_hi = nc.dram_tensor("xT_hi", [D, L], f16, kind="ExternalInput")
    xT_lo = nc.dram_tensor("xT_lo", [D, L], f16, kind="ExternalInput")
    Hm = nc.dram_tensor("Hm", [D, D], f16, kind="ExternalInput")
    WqT = nc.dram_tensor("WqT", [D, P], f16, kind="ExternalInput")
    WkT = nc.dram_tensor("WkT", [D, P], f16, kind="ExternalInput")
    WvT = nc.dram_tensor("WvT", [D, P], f16, kind="ExternalInput")
    WoT = nc.dram_tensor("WoT", [D, D], f16, kind="ExternalInput")
    consts = nc.dram_tensor("consts", [1, 4], f32, kind="ExternalInput")
    z = nc.dram_tensor("z", [L, D], f32, kind="ExternalOutput")

    with tile.TileContext(nc) as tc:
        cpool = tc.alloc_tile_pool(name="cpool", bufs=1)
        dram = tc.alloc_tile_pool(name="dram", bufs=1, space="DRAM")

        ident16 = cpool.tile([P, P], f16)
        make_identity(nc, ident16)
        ident32 = cpool.tile([P, P], f32)
        make_identity(nc, ident32)
        csb = cpool.tile([P, 4], f32)
        nc.sync.dma_start(csb, consts.ap().to_broadcast((P, 4)))

        # DRAM intermediates
        yT_loc0 = dram.tile([D, L // 2], i8)
        yT_loc1 = dram.tile([D, L // 2], i8)
        yT_g0 = dram.tile([N_CORES * D, L // 2], i8, addr_space="Shared")
        yT_g1 = dram.tile([N_CORES * D, L // 2], i8, addr_space="Shared")
        am_loc = dram.tile([1, L], f32)
        am_g = dram.tile([N_CORES, L], f32, addr_space="Shared")
        a2a_in = dram.tile([N_CORES * P, L], f32)
        a2a_out = dram.tile([N_CORES * P, L], f32)
        y2_loc = dram.tile([L, D], f16)

        # ---------------- Phase A: x@H, quant, transpose, gather ----------
        with tc.tile_pool(name="pre", bufs=1) as pre, \
             tc.tile_pool(name="workA", bufs=3) as workA, \
             tc.tile_pool(name="psA", bufs=2, space="PSUM") as psA, \
             tc.tile_pool(name="psT", bufs=3, space="PSUM") as psT:
            sA = nc.named_scope("phaseA"); sA.__enter__()
            xhi = pre.tile([P, DK, L], f16)
            xlo = pre.tile([P, DK, L], f16)
            Hsb = pre.tile([P, DK, D], f16)
            xhi_v = xT_hi.ap().rearrange("(o p) t -> p o t", p=P)
            xlo_v = xT_lo.ap().rearrange("(o p) t -> p o t", p=P)
            H_v = Hm.ap().rearrange("(o p) d -> p o d", p=P)
            for kc in range(DK):
                nc.sync.dma_start(Hsb[:, kc], H_v[:, kc])
                nc.sync.dma_start(xhi[:, kc], xhi_v[:, kc])
                nc.sync.dma_start(xlo[:, kc], xlo_v[:, kc])
            yT_sb = pre.tile([P, DK, L], i8)
            am_all = pre.tile([P, NT], f32)

            for tt in range(NT):
                ps = psA.tile([P, 1024], f32, tag="xh")
                for half in range(2):
                    for kc in range(DK):
                        nc.tensor.matmul(
                            ps[:, half * 512:(half + 1) * 512],
                            xhi[:, kc, tt * P:(tt + 1) * P],
                            Hsb[:, kc, half * 512:(half + 1) * 512],
                            start=(kc == 0), stop=False)
                        nc.tensor.matmul(
                            ps[:, half * 512:(half + 1) * 512],
                            xlo[:, kc, tt * P:(tt + 1) * P],
                            Hsb[:, kc, half * 512:(half + 1) * 512],
                            start=False, stop=(kc == DK - 1))
                am_t = am_all[:, tt:tt + 1]
                nc.vector.reduce_max(am_t, ps, axis=X, apply_absolute_value=True)
                nc.vector.tensor_scalar_max(am_t, am_t, 1e-5)
                rec = workA.tile([P, 1], f32, tag="rec")
                nc.vector.reciprocal(rec, am_t)
                s127 = workA.tile([P, 1], f32, tag="s127")
                nc.vector.tensor_scalar_mul(s127, rec, 127.0)
                tmp = workA.tile([P, 1024], f32, tag="tmpA")
                nc.vector.tensor_scalar(tmp, ps, s127, MAGIC, mult, add)
                y_t = workA.tile([P, 1024], f16, tag="yA")
                nc.vector.tensor_scalar(y_t, tmp, MAGIC, None, subtract)
                if tt == NT - 1:
                    nc.sync.dma_start(am_loc.rearrange("1 (t p) -> p t", p=P),
                                      am_all)
                    nc.gpsimd.collective_compute(
                        "AllGather", mybir.AluOpType.bypass,
                        replica_groups=GROUPS,
                        ins=[am_loc.opt()], outs=[am_g.opt()])
                for kc in range(DK):
                    pst = psT.tile([P, P], f16, tag="trA")
                    nc.tensor.transpose(pst, y_t[:, kc * P:(kc + 1) * P], ident16)
                    nc.vector.tensor_copy(yT_sb[:, kc, tt * P:(tt + 1) * P], pst)
            sA.__exit__(None, None, None)
            sG = nc.named_scope("gather"); sG.__enter__()
            if NT > 1:
                nc.sync.dma_start(
                    yT_loc0.rearrange("(o p) t -> p o t", p=P),
                    yT_sb[:, :, 0:L // 2])
                nc.gpsimd.collective_compute(
                    "AllGather", mybir.AluOpType.bypass, replica_groups=GROUPS,
                    ins=[yT_loc0.opt()], outs=[yT_g0.opt()])
            else:
                nc.sync.dma_start(
                    yT_loc0.rearrange("(o p) t -> p o t", p=P),
                    yT_sb[:, :, 0:L // 2])
                nc.gpsimd.collective_compute(
                    "AllGather", mybir.AluOpType.bypass, replica_groups=GROUPS,
                    ins=[yT_loc0.opt()], outs=[yT_g0.opt()])
            nc.sync.dma_start(
                yT_loc1.rearrange("(o p) t -> p o t", p=P),
                yT_sb[:, :, L // 2:L])
            nc.gpsimd.collective_compute(
                "AllGather", mybir.AluOpType.bypass, replica_groups=GROUPS,
                ins=[yT_loc1.opt()], outs=[yT_g1.opt()])
            sG.__exit__(None, None, None)

        # ---------------- Phase B: QKV ------------------------------------
        fin = tc.alloc_tile_pool(name="fin", bufs=1)
        attn = tc.alloc_tile_pool(name="attn", bufs=1)
        QsT = attn.tile([P, BT], f16)
        KsT = attn.tile([P, BT], f16)
        V_A = attn.tile([P, VT, 65], f16)
        V_B = attn.tile([P, VT, 65], f16)

        with tc.tile_pool(name="gath", bufs=1) as gath, \
             tc.tile_pool(name="workB", bufs=3) as workB, \
             tc.tile_pool(name="psB", bufs=2, space="PSUM") as psB:
            sB = nc.named_scope("phaseB"); sB.__enter__()
            yTg = gath.tile([P, DK, BT], f16)
            yv0 = yT_g0.rearrange("(a o p) t -> a p o t", p=P, o=DK)
            yv1 = yT_g1.rearrange("(a o p) t -> a p o t", p=P, o=DK)
            for peer in range(N_CORES):
                for half, yv in ((0, yv0), (1, yv1)):
                    stg = workB.tile([P, DK, L // 2], i8, tag="stg")
                    nc.sync.dma_start(stg, yv[peer])
                    base = peer * L + half * (L // 2)
                    nc.vector.tensor_copy(yTg[:, :, base:base + L // 2], stg)

            A_q = gath.tile([P, BT], f32)
            A_k = gath.tile([P, BT], f32)
            nc.sync.dma_start(
                A_k, am_g.rearrange("a l -> (a l)")[None, :].to_broadcast((P, BT)))
            nc.vector.tensor_scalar(A_q, A_k, csb[:, 0:1], None, mult)
            nc.vector.tensor_scalar(A_k, A_k, 1.0 / 127.0, None, mult)
            Av = gath.tile([P, VT], f32)
            nc.sync.dma_start(
                Av, am_g.rearrange("a (t p) -> p (a t)", p=P))
            nc.vector.tensor_scalar(Av, Av, csb[:, 1:2], None, mult)

            wq = gath.tile([P, DK, P], f16)
            nc.sync.dma_start(wq, WqT.ap().rearrange("(o p) m -> p o m", p=P))
            wk = gath.tile([P, DK, P], f16)
            nc.sync.dma_start(wk, WkT.ap().rearrange("(o p) m -> p o m", p=P))
            wv = gath.tile([P, DK, P], f16)
            nc.sync.dma_start(wv, WvT.ap().rearrange("(o p) m -> p o m", p=P))

            TBW = min(512, L // 2)
            for tb in range(BT // TBW):
                sl = slice(tb * TBW, (tb + 1) * TBW)
                psq = psB.tile([P, TBW], f32, tag="psq")
                for kc in range(DK):
                    nc.tensor.matmul(psq, wq[:, kc], yTg[:, kc, sl],
                                     start=(kc == 0), stop=(kc == DK - 1))
                nc.vector.tensor_tensor(QsT[:, sl], psq, A_q[:, sl], mult)
                psk = psB.tile([P, TBW], f32, tag="psk")
                for kc in range(DK):
                    nc.tensor.matmul(psk, wk[:, kc], yTg[:, kc, sl],
                                     start=(kc == 0), stop=(kc == DK - 1))
                nc.vector.tensor_tensor(KsT[:, sl], psk, A_k[:, sl], mult)

            nc.vector.memset(V_A[:, :, 64:65], 1.0)
            nc.vector.memset(V_B[:, :, 64:65], 1.0)
            for vt in range(VT):
                psv = psB.tile([P, P], f32, tag="psv")
                for kc in range(DK):
                    nc.tensor.matmul(psv, yTg[:, kc, vt * P:(vt + 1) * P],
                                     wv[:, kc],
                                     start=(kc == 0), stop=(kc == DK - 1))
                nc.vector.tensor_scalar(V_A[:, vt, 0:64], psv[:, 0:64],
                                        Av[:, vt:vt + 1], None, mult)
                nc.vector.tensor_scalar(V_B[:, vt, 0:64], psv[:, 64:128],
                                        Av[:, vt:vt + 1], None, mult)

        # ---------------- Phase C: attention ------------------------------
        sB.__exit__(None, None, None)
        wo = fin.tile([P, DK, D], f16)
        nc.sync.dma_start(wo, WoT.ap().rearrange("(o p) n -> p o n", p=P))

        with tc.tile_pool(name="workC", bufs=6) as workC, \
             tc.tile_pool(name="dramC", bufs=3, space="DRAM") as dramC, \
             tc.tile_pool(name="psS", bufs=2, space="PSUM") as psS_pool, \
             tc.tile_pool(name="psO", bufs=2, space="PSUM") as psO_pool:
            sC = nc.named_scope("phaseC"); sC.__enter__()
            a2a_in_v = a2a_in.rearrange("(a p) l -> a p l", a=N_CORES)
            for b in range(B):
                for qb in range(NQB):
                    q0 = b * T + qb * QB
                    poA = psO_pool.tile([P, 512], f32, tag="poA")
                    poB = psO_pool.tile([P, 512], f32, tag="poB")
                    for kt in range(NKT):
                        k0 = b * T + kt * P
                        ps = psS_pool.tile([P, 1024], f32, tag="S")
                        nc.tensor.matmul(ps[:, 0:512],
                                         KsT[0:64, k0:k0 + P],
                                         QsT[0:64, q0:q0 + QB],
                                         start=True, stop=True)
                        nc.tensor.matmul(ps[:, 512:1024],
                                         KsT[64:128, k0:k0 + P],
                                         QsT[64:128, q0:q0 + QB],
                                         start=True, stop=True)
                        ex = workC.tile([P, 1024], f16, tag="ex")
                        nc.scalar.activation(ex, ps, Exp)
                        vt = (b * T) // P + kt
                        nc.tensor.matmul(poA[0:65], V_A[:, vt], ex[:, 0:512],
                                         start=(kt == 0), stop=(kt == NKT - 1))
                        nc.tensor.matmul(poB[0:65], V_B[:, vt], ex[:, 512:1024],
                                         start=(kt == 0), stop=(kt == NKT - 1))
                    for head, po in ((0, poA), (1, poB)):
                        rrow = workC.tile([1, QB], f32, tag="rrow")
                        nc.vector.reciprocal(rrow, po[64:65, 0:QB])
                        rdr = dramC.tile([1, QB], f32, tag="rdr")
                        nc.sync.dma_start(rdr, rrow)
                        rbc = workC.tile([64, QB], f32, tag="rbc")
                        nc.sync.dma_start(rbc, rdr.to_broadcast((64, QB)))
                        onrm = workC.tile([64, QB], f32, tag="onrm")
                        nc.vector.tensor_tensor(onrm, po[0:64], rbc, mult)
                        step = min(QB, L)
                        for j in range(QB // step):
                            peer, tl = divmod(q0 + j * step, L)
                            nc.sync.dma_start(
                                a2a_in_v[peer, head * 64:(head + 1) * 64,
                                         tl:tl + step],
                                onrm[:, j * step:(j + 1) * step])

            sC.__exit__(None, None, None)
            sA2A = nc.named_scope("a2a"); sA2A.__enter__()
            nc.gpsimd.collective_compute(
                "AllToAll", mybir.AluOpType.bypass, replica_groups=GROUPS,
                ins=[a2a_in.opt()], outs=[a2a_out.opt()])
            sA2A.__exit__(None, None, None)

        attn.release()

        # ---------------- Phase D: final quant + output projection --------
        with tc.tile_pool(name="workD", bufs=3) as workD, \
             tc.tile_pool(name="finD", bufs=1) as finD, \
             tc.tile_pool(name="psD", bufs=2, space="PSUM") as psD, \
             tc.tile_pool(name="psTD", bufs=3, space="PSUM") as psTD:
            sD = nc.named_scope("phaseD"); sD.__enter__()
            outT = finD.tile([P, DK, L], f32)
            a2a_out_v = a2a_out.rearrange("(a p) l -> a p l", a=N_CORES)
            for peer in range(N_CORES):
                nc.sync.dma_start(outT[:, peer, :], a2a_out_v[peer])
            outf = finD.tile([P, NT, D], f32)
            for tt in range(NT):
                for o in range(DK):
                    pst = psTD.tile([P, P], f32, tag="trD2")
                    nc.tensor.transpose(pst, outT[:, o, tt * P:(tt + 1) * P],
                                        ident32)
                    nc.vector.tensor_copy(outf[:, tt, o * P:(o + 1) * P], pst)
            y2T = finD.tile([P, DK, L], f16)
            a2r = finD.tile([P, NT], f32)
            for tt in range(NT):
                am2 = workD.tile([P, 1], f32, tag="am2")
                nc.vector.reduce_max(am2, outf[:, tt], axis=X,
                                     apply_absolute_value=True)
                nc.vector.tensor_scalar_max(am2, am2, 1e-5)
                nc.vector.tensor_tensor(a2r[:, tt:tt + 1], am2, csb[:, 2:3], mult)
                rec = workD.tile([P, 1], f32, tag="recD")
                nc.vector.reciprocal(rec, am2)
                s127 = workD.tile([P, 1], f32, tag="s127D")
                nc.vector.tensor_scalar_mul(s127, rec, 127.0)
                tmp = workD.tile([P, D], f32, tag="tmpD")
                nc.vector.tensor_scalar(tmp, outf[:, tt], s127, MAGIC, mult, add)
                y2 = workD.tile([P, D], f16, tag="y2")
                nc.vector.tensor_scalar(y2, tmp, MAGIC, None, subtract)
                for kc in range(DK):
                    pst = psTD.tile([P, P], f16, tag="trD")
                    nc.tensor.transpose(pst, y2[:, kc * P:(kc + 1) * P], ident16)
                    nc.vector.tensor_copy(y2T[:, kc, tt * P:(tt + 1) * P], pst)
            for tt in range(NT):
                for nh in range(2):
                    psz = psD.tile([P, 512], f32, tag="psz")
                    for kc in range(DK):
                        nc.tensor.matmul(psz, y2T[:, kc, tt * P:(tt + 1) * P],
                                         wo[:, kc, nh * 512:(nh + 1) * 512],
                                         start=(kc == 0), stop=(kc == DK - 1))
                    zsb = workD.tile([P, 512], f32, tag="zsb")
                    nc.vector.tensor_scalar(zsb, psz, a2r[:, tt:tt + 1], None,
                                            mult)
                    nc.sync.dma_start(
                        z.ap()[tt * P:(tt + 1) * P, nh * 512:(nh + 1) * 512],
                        zsb)

        sD.__exit__(None, None, None)
        fin.release()
        dram.release()
        cpool.release()

    nc.compile()
    return nc


def _get_nc(T):
    if T not in _BUILD_CACHE:
        _BUILD_CACHE[T] = _build(T)
    return _BUILD_CACHE[T]


def _wquant(w):
    # reference: scale = 1/clip(mean|w|,1e-5); u = clip(round(w*scale),-1,1)/scale
    scale = np.float32(1.0) / np.maximum(
        np.float32(np.mean(np.abs(w), dtype=np.float64)), np.float32(1e-5))
    u = np.clip(np.rint(w * scale), -1, 1).astype(np.float32)
    return u, np.float32(1.0) / scale  # ternary, dequant scale (= clipped mean)


def kernel(x, mask, Wq, Wk, Wv, Wo, H):
    from concourse.bass_utils import run_bass_kernel_spmd

    x = np.asarray(x, np.float32)
    Wq = np.asarray(Wq, np.float32); Wk = np.asarray(Wk, np.float32)
    Wv = np.asarray(Wv, np.float32); Wo = np.asarray(Wo, np.float32)
    H = np.asarray(H, np.float32)
    Bx, T, Dx = x.shape
    BT = Bx * T
    L = BT // N_CORES

    nc = _get_nc(T)

    xf = x.reshape(BT, Dx)
    x_hi = xf.astype(np.float16)
    x_lo = (xf - x_hi.astype(np.float32)).astype(np.float16)
    H16 = H.astype(np.float16)

    uq, cq = _wquant(Wq); uk, ck = _wquant(Wk)
    uv, cv = _wquant(Wv); uo, co = _wquant(Wo)
    uqT = np.ascontiguousarray(uq.T.astype(np.float16))
    ukT = np.ascontiguousarray(uk.T.astype(np.float16))
    uvT = np.ascontiguousarray(uv.T.astype(np.float16))
    uoT = np.ascontiguousarray(uo.T.astype(np.float16))

    c0 = np.float32(cq) * np.float32(ck) / (np.float32(np.sqrt(DH)) * np.float32(127.0))
    c1 = np.float32(cv) / np.float32(127.0)
    c2 = np.float32(co) / np.float32(127.0)
    consts = np.array([[c0, c1, c2, 0.0]], np.float32)

    in_maps = []
    for c in range(N_CORES):
        rows = slice(c * L, (c + 1) * L)
        cols = slice(c * P, (c + 1) * P)
        in_maps.append({
            "xT_hi": np.ascontiguousarray(x_hi[rows].T),
            "xT_lo": np.ascontiguousarray(x_lo[rows].T),
            "Hm": H16,
            "WqT": np.ascontiguousarray(uqT[:, cols]),
            "WkT": np.ascontiguousarray(ukT[:, cols]),
            "WvT": np.ascontiguousarray(uvT[:, cols]),
            "WoT": uoT,
            "consts": consts,
        })

    res = run_bass_kernel_spmd(nc, in_maps, core_ids=list(range(N_CORES)))
    kernel.last_results = res
    z = np.concatenate([res.results[c]["z"] for c in range(N_CORES)], axis=0)
    return z.reshape(Bx, T, Dx).astype(np.float32)


# revision 21
# speedup vs baseline: 1.0484x; 1.0024x over previous
"""BitNet attention TRN2 kernel: 8-way SPMD (2 heads/core, tokens sharded 8-way).

Dataflow per core c (tokens Tc = rows [c*L,(c+1)*L) of the flattened [B*T, D]
activations, heads {2c, 2c+1}):
  A) x_had = x @ H  via fp16 hi/lo split matmuls (fp32 accumulate);
     per-token int8 absmax quant -> y (fp16-held small ints); PE-transpose;
     AllGather y.T and the per-token absmax across the 8 cores.
  B) Q/K/V projections in the integer domain (exact in fp16), dequantized with
     per-token scales (DMA-broadcast absmax vector x host weight-scale consts).
  C) Per (batch, head): S.T = Ks.T^T @ Qs.T (row-packed head pairs), exp on ACT,
     out.T = [V | 1]^T @ expS.T accumulated over key tiles (ones column gives the
     softmax denominator), normalize, PE-transpose to token-major, AllToAll so
     each core gets its own tokens x all 1024 columns.
  D) Second absmax quant, z = y2 @ Wo_u.T (integer domain), per-token dequant.

Host side quantizes weights to ternary {-1,0,1} (fp16-exact), splits/transposes
x, and concatenates the 8 z slices.
"""
import sys

if "/opt/trn_rl_repo" not in sys.path:
    sys.path.insert(0, "/opt/trn_rl_repo")

import numpy as np

P = 128
D = 1024
NH = 16
DH = 64
B = 2
N_CORES = 8
MAGIC = 12582912.0  # 1.5 * 2**23: fp32 round-to-nearest-int via add/sub

_BUILD_CACHE = {}


def _build(T):
    import concourse.bass as bass  # noqa: F401
    import concourse.mybir as mybir
    import concourse.tile as tile
    from concourse import bacc
    from concourse.masks import make_identity

    f16 = mybir.dt.float16
    f32 = mybir.dt.float32
    i8 = mybir.dt.int8
    Exp = mybir.ActivationFunctionType.Exp
    mult = mybir.AluOpType.mult
    add = mybir.AluOpType.add
    subtract = mybir.AluOpType.subtract
    X = mybir.AxisListType.X

    BT = B * T
    L = BT // N_CORES          # tokens per core
    NT = L // P                # local token tiles
    DK = D // P                # contraction chunks
    QB = 512                   # query block
    NQB = T // QB              # query blocks per batch
    NKT = T // P               # key tiles per batch
    VT = BT // P               # global token tiles (for V)
    GROUPS = [list(range(N_CORES))]

    nc = bacc.Bacc("TRN2", target_bir_lowering=False, debug=False,
                   num_devices=N_CORES)

    # I/O
    xT_hi = nc.dram_tensor("xT_hi", [D, L], f16, kind="ExternalInput")
    xT_lo = nc.dram_tensor("xT_lo", [D, L], f16, kind="ExternalInput")
    Hm = nc.dram_tensor("Hm", [D, D], f16, kind="ExternalInput")
    WqT = nc.dram_tensor("WqT", [D, P], f16, kind="ExternalInput")
    WkT = nc.dram_tensor("WkT", [D, P], f16, kind="ExternalInput")
    WvT = nc.dram_tensor("WvT", [D, P], f16, kind="ExternalInput")
    WoT = nc.dram_tensor("WoT", [D, D], f16, kind="ExternalInput")
    consts = nc.dram_tensor("consts", [1, 4], f32, kind="ExternalInput")
    z = nc.dram_tensor("z", [L, D], f32, kind="ExternalOutput")

    with tile.TileContext(nc) as tc:
        cpool = tc.alloc_tile_pool(name="cpool", bufs=1)
        dram = tc.alloc_tile_pool(name="dram", bufs=1, space="DRAM")

        ident16 = cpool.tile([P, P], f16)
        make_identity(nc, ident16)
        ident32 = cpool.tile([P, P], f32)
        make_identity(nc, ident32)
        csb = cpool.tile([P, 4], f32)
        nc.sync.dma_start(csb, consts.ap().to_broadcast((P, 4)))

        # DRAM intermediates
        yT_loc0 = dram.tile([D, L // 2], i8)
        yT_loc1 = dram.tile([D, L // 2], i8)
        yT_g0 = dram.tile([N_CORES * D, L // 2], i8, addr_space="Shared")
        yT_g1 = dram.tile([N_CORES * D, L // 2], i8, addr_space="Shared")
        am_loc = dram.tile([1, L], f32)
        am_g = dram.tile([N_CORES, L], f32, addr_space="Shared")
        a2a_in = dram.tile([N_CORES * P, L], f32)
        a2a_out = dram.tile([N_CORES * P, L], f32)
        y2_loc = dram.tile([L, D], f16)

        # ---------------- Phase A: x@H, quant, transpose, gather ----------
        with tc.tile_pool(name="pre", bufs=1) as pre, \
             tc.tile_pool(name="workA", bufs=3) as workA, \
             tc.tile_pool(name="psA", bufs=2, space="PSUM") as psA, \
             tc.tile_pool(name="psT", bufs=4, space="PSUM") as psT:
            sA = nc.named_scope("phaseA"); sA.__enter__()
            xhi = pre.tile([P, DK, L], f16)
            xlo = pre.tile([P, DK, L], f16)
            Hsb = pre.tile([P, DK, D], f16)
            xhi_v = xT_hi.ap().rearrange("(o p) t -> p o t", p=P)
            xlo_v = xT_lo.ap().rearrange("(o p) t -> p o t", p=P)
            H_v = Hm.ap().rearrange("(o p) d -> p o d", p=P)
            for kc in range(DK):
                nc.sync.dma_start(Hsb[:, kc], H_v[:, kc])
                nc.sync.dma_start(xhi[:, kc], xhi_v[:, kc])
                nc.sync.dma_start(xlo[:, kc], xlo_v[:, kc])
            yT_sb = pre.tile([P, DK, L], i8)
            am_all = pre.tile([P, NT], f32)

            for tt in range(NT):
                ps = psA.tile([P, 1024], f32, tag="xh")
                for half in range(2):
                    for kc in range(DK):
                        nc.tensor.matmul(
                            ps[:, half * 512:(half + 1) * 512],
                            xhi[:, kc, tt * P:(tt + 1) * P],
                            Hsb[:, kc, half * 512:(half + 1) * 512],
                            start=(kc == 0), stop=False)
                        nc.tensor.matmul(
                            ps[:, half * 512:(half + 1) * 512],
                            xlo[:, kc, tt * P:(tt + 1) * P],
                            Hsb[:, kc, half * 512:(half + 1) * 512],
                            start=False, stop=(kc == DK - 1))
                am_t = am_all[:, tt:tt + 1]
                nc.vector.reduce_max(am_t, ps, axis=X, apply_absolute_value=True)
                nc.vector.tensor_scalar_max(am_t, am_t, 1e-5)
                rec = workA.tile([P, 1], f32, tag="rec")
                nc.vector.reciprocal(rec, am_t)
                s127 = workA.tile([P, 1], f32, tag="s127")
                nc.vector.tensor_scalar_mul(s127, rec, 127.0)
                tmp = workA.tile([P, 1024], f32, tag="tmpA")
                nc.vector.tensor_scalar(tmp, ps, s127, MAGIC, mult, add)
                y_t = workA.tile([P, 1024], f16, tag="yA")
                nc.vector.tensor_scalar(y_t, tmp, MAGIC, None, subtract)
                if tt == NT - 1:
                    nc.sync.dma_start(am_loc.rearrange("1 (t p) -> p t", p=P),
                                      am_all)
                    nc.gpsimd.collective_compute(
                        "AllGather", mybir.AluOpType.bypass,
                        replica_groups=GROUPS,
                        ins=[am_loc.opt()], outs=[am_g.opt()])
                for kc in range(DK):
                    pst = psT.tile([P, P], f16, tag="trA")
                    nc.tensor.transpose(pst, y_t[:, kc * P:(kc + 1) * P], ident16)
                    nc.vector.tensor_copy(yT_sb[:, kc, tt * P:(tt + 1) * P], pst)
            sA.__exit__(None, None, None)
            sG = nc.named_scope("gather"); sG.__enter__()
            if NT > 1:
                nc.sync.dma_start(
                    yT_loc0.rearrange("(o p) t -> p o t", p=P),
                    yT_sb[:, :, 0:L // 2])
                nc.gpsimd.collective_compute(
                    "AllGather", mybir.AluOpType.bypass, replica_groups=GROUPS,
                    ins=[yT_loc0.opt()], outs=[yT_g0.opt()])
            else:
                nc.sync.dma_start(
                    yT_loc0.rearrange("(o p) t -> p o t", p=P),
                    yT_sb[:, :, 0:L // 2])
                nc.gpsimd.collective_compute(
                    "AllGather", mybir.AluOpType.bypass, replica_groups=GROUPS,
                    ins=[yT_loc0.opt()], outs=[yT_g0.opt()])
            nc.sync.dma_start(
                yT_loc1.rearrange("(o p) t -> p o t", p=P),
                yT_sb[:, :, L // 2:L])
            nc.gpsimd.collective_compute(
                "AllGather", mybir.AluOpType.bypass, replica_groups=GROUPS,
                ins=[yT_loc1.opt()], outs=[yT_g1.opt()])
            sG.__exit__(None, None, None)

        # ---------------- Phase B: QKV ------------------------------------
        fin = tc.alloc_tile_pool(name="fin", bufs=1)
        attn = tc.alloc_tile_pool(name="attn", bufs=1)
        QsT = attn.tile([P, BT], f16)
        KsT = attn.tile([P, BT], f16)
        V_A = attn.tile([P, VT, 65], f16)
        V_B = attn.tile([P, VT, 65], f16)

        with tc.tile_pool(name="gath", bufs=1) as gath, \
             tc.tile_pool(name="workB", bufs=4) as workB, \
             tc.tile_pool(name="psB", bufs=2, space="PSUM") as psB:
            sB = nc.named_scope("phaseB"); sB.__enter__()
            yTg = gath.tile([P, DK, BT], f16)
            yv0 = yT_g0.rearrange("(a o p) t -> a p o t", p=P, o=DK)
            yv1 = yT_g1.rearrange("(a o p) t -> a p o t", p=P, o=DK)
            for peer in range(N_CORES):
                for half, yv in ((0, yv0), (1, yv1)):
                    stg = workB.tile([P, DK, L // 2], i8, tag="stg")
                    nc.sync.dma_start(stg, yv[peer])
                    base = peer * L + half * (L // 2)
                    nc.vector.tensor_copy(yTg[:, :, base:base + L // 2], stg)

            A_q = gath.tile([P, BT], f32)
            A_k = gath.tile([P, BT], f32)
            nc.sync.dma_start(
                A_k, am_g.rearrange("a l -> (a l)")[None, :].to_broadcast((P, BT)))
            nc.vector.tensor_scalar(A_q, A_k, csb[:, 0:1], None, mult)
            nc.vector.tensor_scalar(A_k, A_k, 1.0 / 127.0, None, mult)
            Av = gath.tile([P, VT], f32)
            nc.sync.dma_start(
                Av, am_g.rearrange("a (t p) -> p (a t)", p=P))
            nc.vector.tensor_scalar(Av, Av, csb[:, 1:2], None, mult)

            wq = gath.tile([P, DK, P], f16)
            nc.sync.dma_start(wq, WqT.ap().rearrange("(o p) m -> p o m", p=P))
            wk = gath.tile([P, DK, P], f16)
            nc.sync.dma_start(wk, WkT.ap().rearrange("(o p) m -> p o m", p=P))
            wv = gath.tile([P, DK, P], f16)
            nc.sync.dma_start(wv, WvT.ap().rearrange("(o p) m -> p o m", p=P))

            TBW = min(512, L // 2)
            for tb in range(BT // TBW):
                sl = slice(tb * TBW, (tb + 1) * TBW)
                psq = psB.tile([P, TBW], f32, tag="psq")
                for kc in range(DK):
                    nc.tensor.matmul(psq, wq[:, kc], yTg[:, kc, sl],
                                     start=(kc == 0), stop=(kc == DK - 1))
                nc.vector.tensor_tensor(QsT[:, sl], psq, A_q[:, sl], mult)
                psk = psB.tile([P, TBW], f32, tag="psk")
                for kc in range(DK):
                    nc.tensor.matmul(psk, wk[:, kc], yTg[:, kc, sl],
                                     start=(kc == 0), stop=(kc == DK - 1))
                nc.vector.tensor_tensor(KsT[:, sl], psk, A_k[:, sl], mult)

            nc.vector.memset(V_A[:, :, 64:65], 1.0)
            nc.vector.memset(V_B[:, :, 64:65], 1.0)
            for vt in range(VT):
                psv = psB.tile([P, P], f32, tag="psv")
                for kc in range(DK):
                    nc.tensor.matmul(psv, yTg[:, kc, vt * P:(vt + 1) * P],
                                     wv[:, kc],
                                     start=(kc == 0), stop=(kc == DK - 1))
                nc.vector.tensor_scalar(V_A[:, vt, 0:64], psv[:, 0:64],
                                        Av[:, vt:vt + 1], None, mult)
                nc.vector.tensor_scalar(V_B[:, vt, 0:64], psv[:, 64:128],
                                        Av[:, vt:vt + 1], None, mult)

        # ---------------- Phase C: attention ------------------------------
        sB.__exit__(None, None, None)
        wo = fin.tile([P, DK, D], f16)
        nc.sync.dma_start(wo, WoT.ap().rearrange("(o p) n -> p o n", p=P))

        with tc.tile_pool(name="workC", bufs=6) as workC, \
             tc.tile_pool(name="dramC", bufs=3, space="DRAM") as dramC, \
             tc.tile_pool(name="psS", bufs=2, space="PSUM") as psS_pool, \
             tc.tile_pool(name="psO", bufs=2, space="PSUM") as psO_pool:
            sC = nc.named_scope("phaseC"); sC.__enter__()
            a2a_in_v = a2a_in.rearrange("(a p) l -> a p l", a=N_CORES)
            for b in range(B):
                for qb in range(NQB):
                    q0 = b * T + qb * QB
                    poA = psO_pool.tile([P, 512], f32, tag="poA")
                    poB = psO_pool.tile([P, 512], f32, tag="poB")
                    for kt in range(NKT):
                        k0 = b * T + kt * P
                        ps = psS_pool.tile([P, 1024], f32, tag="S")
                        nc.tensor.matmul(ps[:, 0:512],
                                         KsT[0:64, k0:k0 + P],
                                         QsT[0:64, q0:q0 + QB],
                                         start=True, stop=True)
                        nc.tensor.matmul(ps[:, 512:1024],
                                         KsT[64:128, k0:k0 + P],
                                         QsT[64:128, q0:q0 + QB],
                                         start=True, stop=True)
                        ex = workC.tile([P, 1024], f16, tag="ex")
                        nc.scalar.activation(ex, ps, Exp)
                        vt = (b * T) // P + kt
                        nc.tensor.matmul(poA[0:65], V_A[:, vt], ex[:, 0:512],
                                         start=(kt == 0), stop=(kt == NKT - 1))
                        nc.tensor.matmul(poB[0:65], V_B[:, vt], ex[:, 512:1024],
                                         start=(kt == 0), stop=(kt == NKT - 1))
                    for head, po in ((0, poA), (1, poB)):
                        rrow = workC.tile([1, QB], f32, tag="rrow")
                        nc.vector.reciprocal(rrow, po[64:65, 0:QB])
                        rdr = dramC.tile([1, QB], f32, tag="rdr")
                        nc.sync.dma_start(rdr, rrow)
                        rbc = workC.tile([64, QB], f32, tag="rbc")
                        nc.sync.dma_start(rbc, rdr.to_broadcast((64, QB)))
                        onrm = workC.tile([64, QB], f32, tag="onrm")
                        nc.vector.tensor_tensor(onrm, po[0:64], rbc, mult)
                        step = min(QB, L)
                        for j in range(QB // step):
                            peer, tl = divmod(q0 + j * step, L)
                            nc.sync.dma_start(
                                a2a_in_v[peer, head * 64:(head + 1) * 64,
                                         tl:tl + step],
                                onrm[:, j * step:(j + 1) * step])

            sC.__exit__(None, None, None)
            sA2A = nc.named_scope("a2a"); sA2A.__enter__()
            nc.gpsimd.collective_compute(
                "AllToAll", mybir.AluOpType.bypass, replica_groups=GROUPS,
                ins=[a2a_in.opt()], outs=[a2a_out.opt()])
            sA2A.__exit__(None, None, None)

        attn.release()

        # ---------------- Phase D: final quant + output projection --------
        with tc.tile_pool(name="workD", bufs=4) as workD, \
             tc.tile_pool(name="finD", bufs=1) as finD, \
             tc.tile_pool(name="psD", bufs=2, space="PSUM") as psD, \
             tc.tile_pool(name="psTD", bufs=2, space="PSUM") as psTD:
            sD = nc.named_scope("phaseD"); sD.__enter__()
            outT = finD.tile([P, DK, L], f32)
            a2a_out_v = a2a_out.rearrange("(a p) l -> a p l", a=N_CORES)
            for peer in range(N_CORES):
                nc.sync.dma_start(outT[:, peer, :], a2a_out_v[peer])
            outf = finD.tile([P, NT, D], f32)
            for tt in range(NT):
                for o in range(DK):
                    pst = psTD.tile([P, P], f32, tag="trD2")
                    nc.tensor.transpose(pst, outT[:, o, tt * P:(tt + 1) * P],
                                        ident32)
                    nc.vector.tensor_copy(outf[:, tt, o * P:(o + 1) * P], pst)
            y2T = finD.tile([P, DK, L], f16)
            a2r = finD.tile([P, NT], f32)
            for tt in range(NT):
                am2 = workD.tile([P, 1], f32, tag="am2")
                nc.vector.reduce_max(am2, outf[:, tt], axis=X,
                                     apply_absolute_value=True)
                nc.vector.tensor_scalar_max(am2, am2, 1e-5)
                nc.vector.tensor_tensor(a2r[:, tt:tt + 1], am2, csb[:, 2:3], mult)
                rec = workD.tile([P, 1], f32, tag="recD")
                nc.vector.reciprocal(rec, am2)
                s127 = workD.tile([P, 1], f32, tag="s127D")
                nc.vector.tensor_scalar_mul(s127, rec, 127.0)
                tmp = workD.tile([P, D], f32, tag="tmpD")
                nc.vector.tensor_scalar(tmp, outf[:, tt], s127, MAGIC, mult, add)
                y2 = workD.tile([P, D], f16, tag="y2")
                nc.vector.tensor_scalar(y2, tmp, MAGIC, None, subtract)
                for kc in range(DK):
                    pst = psTD.tile([P, P], f16, tag="trD")
                    nc.tensor.transpose(pst, y2[:, kc * P:(kc + 1) * P], ident16)
                    nc.vector.tensor_copy(y2T[:, kc, tt * P:(tt + 1) * P], pst)
            for tt in range(NT):
                for nh in range(2):
                    psz = psD.tile([P, 512], f32, tag="psz")
                    for kc in range(DK):
                        nc.tensor.matmul(psz, y2T[:, kc, tt * P:(tt + 1) * P],
                                         wo[:, kc, nh * 512:(nh + 1) * 512],
                                         start=(kc == 0), stop=(kc == DK - 1))
                    zsb = workD.tile([P, 512], f32, tag="zsb")
                    nc.vector.tensor_scalar(zsb, psz, a2r[:, tt:tt + 1], None,
                                            mult)
                    nc.sync.dma_start(
                        z.ap()[tt * P:(tt + 1) * P, nh * 512:(nh + 1) * 512],
                        zsb)

        sD.__exit__(None, None, None)
        fin.release()
        dram.release()
        cpool.release()

    nc.compile()
    return nc


def _get_nc(T):
    if T not in _BUILD_CACHE:
        _BUILD_CACHE[T] = _build(T)
    return _BUILD_CACHE[T]


def _wquant(w):
    # reference: scale = 1/clip(mean|w|,1e-5); u = clip(round(w*scale),-1,1)/scale
    scale = np.float32(1.0) / np.maximum(
        np.float32(np.mean(np.abs(w), dtype=np.float64)), np.float32(1e-5))
    u = np.clip(np.rint(w * scale), -1, 1).astype(np.float32)
    return u, np.float32(1.0) / scale  # ternary, dequant scale (= clipped mean)


def kernel(x, mask, Wq, Wk, Wv, Wo, H):
    from concourse.bass_utils import run_bass_kernel_spmd

    x = np.asarray(x, np.float32)
    Wq = np.asarray(Wq, np.float32); Wk = np.asarray(Wk, np.float32)
    Wv = np.asarray(Wv, np.float32); Wo = np.asarray(Wo, np.float32)
    H = np.asarray(H, np.float32)
    Bx, T, Dx = x.shape
    BT = Bx * T
    L = BT // N_CORES

    nc = _get_nc(T)

    xf = x.reshape(BT, Dx)
    x_hi = xf.astype(np.float16)
    x_lo = (xf - x_hi.astype(np.float32)).astype(np.float16)
    H16 = H.astype(np.float16)

    uq, cq = _wquant(Wq); uk, ck = _wquant(Wk)
    uv, cv = _wquant(Wv); uo, co = _wquant(Wo)
    uqT = np.ascontiguousarray(uq.T.astype(np.float16))
    ukT = np.ascontiguousarray(uk.T.astype(np.float16))
    uvT = np.ascontiguousarray(uv.T.astype(np.float16))
    uoT = np.ascontiguousarray(uo.T.astype(np.float16))

    c0 = np.float32(cq) * np.float32(ck) / (np.float32(np.sqrt(DH)) * np.float32(127.0))
    c1 = np.float32(cv) / np.float32(127.0)
    c2 = np.float32(co) / np.float32(127.0)
    consts = np.array([[c0, c1, c2, 0.0]], np.float32)

    in_maps = []
    for c in range(N_CORES):
        rows = slice(c * L, (c + 1) * L)
        cols = slice(c * P, (c + 1) * P)
        in_maps.append({
            "xT_hi": np.ascontiguousarray(x_hi[rows].T),
            "xT_lo": np.ascontiguousarray(x_lo[rows].T),
            "Hm": H16,
            "WqT": np.ascontiguousarray(uqT[:, cols]),
            "WkT": np.ascontiguousarray(ukT[:, cols]),
            "WvT": np.ascontiguousarray(uvT[:, cols]),
            "WoT": uoT,
            "consts": consts,
        })

    res = run_bass_kernel_spmd(nc, in_maps, core_ids=list(range(N_CORES)))
    kernel.last_results = res
    z = np.concatenate([res.results[c]["z"] for c in range(N_CORES)], axis=0)
    return z.reshape(Bx, T, Dx).astype(np.float32)


# revision 23
# speedup vs baseline: 1.0616x; 1.0126x over previous
"""BitNet attention TRN2 kernel: 8-way SPMD (2 heads/core, tokens sharded 8-way).

Dataflow per core c (tokens Tc = rows [c*L,(c+1)*L) of the flattened [B*T, D]
activations, heads {2c, 2c+1}):
  A) x_had = x @ H  via fp16 hi/lo split matmuls (fp32 accumulate);
     per-token int8 absmax quant -> y (fp16-held small ints); PE-transpose;
     AllGather y.T and the per-token absmax across the 8 cores.
  B) Q/K/V projections in the integer domain (exact in fp16), dequantized with
     per-token scales (DMA-broadcast absmax vector x host weight-scale consts).
  C) Per (batch, head): S.T = Ks.T^T @ Qs.T (row-packed head pairs), exp on ACT,
     out.T = [V | 1]^T @ expS.T accumulated over key tiles (ones column gives the
     softmax denominator), normalize, PE-transpose to token-major, AllToAll so
     each core gets its own tokens x all 1024 columns.
  D) Second absmax quant, z = y2 @ Wo_u.T (integer domain), per-token dequant.

Host side quantizes weights to ternary {-1,0,1} (fp16-exact), splits/transposes
x, and concatenates the 8 z slices.
"""
import sys

if "/opt/trn_rl_repo" not in sys.path:
    sys.path.insert(0, "/opt/trn_rl_repo")

import numpy as np

P = 128
D = 1024
NH = 16
DH = 64
B = 2
N_CORES = 8
MAGIC = 12582912.0  # 1.5 * 2**23: fp32 round-to-nearest-int via add/sub

_BUILD_CACHE = {}


def _build(T):
    import concourse.bass as bass  # noqa: F401
    import concourse.mybir as mybir
    import concourse.tile as tile
    from concourse import bacc
    from concourse.masks import make_identity

    f16 = mybir.dt.float16
    f32 = mybir.dt.float32
    i8 = mybir.dt.int8
    Exp = mybir.ActivationFunctionType.Exp
    mult = mybir.AluOpType.mult
    add = mybir.AluOpType.add
    subtract = mybir.AluOpType.subtract
    X = mybir.AxisListType.X

    BT = B * T
    L = BT // N_CORES          # tokens per core
    NT = L // P                # local token tiles
    DK = D // P                # contraction chunks
    QB = 512                   # query block
    NQB = T // QB              # query blocks per batch
    NKT = T // P               # key tiles per batch
    VT = BT // P               # global token tiles (for V)
    GROUPS = [list(range(N_CORES))]

    nc = bacc.Bacc("TRN2", target_bir_lowering=False, debug=False,
                   num_devices=N_CORES)

    # I/O
    xT_hi = nc.dram_tensor("xT_hi", [D, L], f16, kind="ExternalInput")
    xT_lo = nc.dram_tensor("xT_lo", [D, L], f16, kind="ExternalInput")
    Hm = nc.dram_tensor("Hm", [D, D], f16, kind="ExternalInput")
    WqT = nc.dram_tensor("WqT", [D, P], f16, kind="ExternalInput")
    WkT = nc.dram_tensor("WkT", [D, P], f16, kind="ExternalInput")
    WvT = nc.dram_tensor("WvT", [D, P], f16, kind="ExternalInput")
    WoT = nc.dram_tensor("WoT", [D, D], f16, kind="ExternalInput")
    consts = nc.dram_tensor("consts", [1, 4], f32, kind="ExternalInput")
    z = nc.dram_tensor("z", [L, D], f32, kind="ExternalOutput")

    with tile.TileContext(nc) as tc:
        cpool = tc.alloc_tile_pool(name="cpool", bufs=1)
        dram = tc.alloc_tile_pool(name="dram", bufs=1, space="DRAM")

        ident16 = cpool.tile([P, P], f16)
        make_identity(nc, ident16)
        ident32 = cpool.tile([P, P], f32)
        make_identity(nc, ident32)
        csb = cpool.tile([P, 4], f32)
        nc.sync.dma_start(csb, consts.ap().to_broadcast((P, 4)))

        # DRAM intermediates
        yT_loc0 = dram.tile([D, L // 2], i8)
        yT_loc1 = dram.tile([D, L // 2], i8)
        yT_g0 = dram.tile([N_CORES * D, L // 2], i8, addr_space="Shared")
        yT_g1 = dram.tile([N_CORES * D, L // 2], i8, addr_space="Shared")
        am_loc = dram.tile([1, L], f32)
        am_g = dram.tile([N_CORES, L], f32, addr_space="Shared")
        a2a_in = dram.tile([N_CORES * P, L], f32)
        a2a_out = dram.tile([N_CORES * P, L], f32)
        y2_loc = dram.tile([L, D], f16)

        # ---------------- Phase A: x@H, quant, transpose, gather ----------
        with tc.tile_pool(name="pre", bufs=1) as pre, \
             tc.tile_pool(name="workA", bufs=3) as workA, \
             tc.tile_pool(name="psA", bufs=2, space="PSUM") as psA, \
             tc.tile_pool(name="psT", bufs=4, space="PSUM") as psT:
            sA = nc.named_scope("phaseA"); sA.__enter__()
            xhi = pre.tile([P, DK, L], f16)
            xlo = pre.tile([P, DK, L], f16)
            Hsb = pre.tile([P, DK, D], f16)
            xhi_v = xT_hi.ap().rearrange("(o p) t -> p o t", p=P)
            xlo_v = xT_lo.ap().rearrange("(o p) t -> p o t", p=P)
            H_v = Hm.ap().rearrange("(o p) d -> p o d", p=P)
            for kc in range(DK):
                nc.sync.dma_start(Hsb[:, kc], H_v[:, kc])
                nc.sync.dma_start(xhi[:, kc], xhi_v[:, kc])
                nc.sync.dma_start(xlo[:, kc], xlo_v[:, kc])
            yT_sb = pre.tile([P, DK, L], i8)
            am_all = pre.tile([P, NT], f32)

            for tt in range(NT):
                ps = psA.tile([P, 1024], f32, tag="xh")
                for half in range(2):
                    for kc in range(DK):
                        nc.tensor.matmul(
                            ps[:, half * 512:(half + 1) * 512],
                            xhi[:, kc, tt * P:(tt + 1) * P],
                            Hsb[:, kc, half * 512:(half + 1) * 512],
                            start=(kc == 0), stop=False)
                        nc.tensor.matmul(
                            ps[:, half * 512:(half + 1) * 512],
                            xlo[:, kc, tt * P:(tt + 1) * P],
                            Hsb[:, kc, half * 512:(half + 1) * 512],
                            start=False, stop=(kc == DK - 1))
                am_t = am_all[:, tt:tt + 1]
                nc.vector.reduce_max(am_t, ps, axis=X, apply_absolute_value=True)
                nc.vector.tensor_scalar_max(am_t, am_t, 1e-5)
                rec = workA.tile([P, 1], f32, tag="rec")
                nc.vector.reciprocal(rec, am_t)
                s127 = workA.tile([P, 1], f32, tag="s127")
                nc.vector.tensor_scalar_mul(s127, rec, 127.0)
                tmp = workA.tile([P, 1024], f32, tag="tmpA")
                nc.vector.tensor_scalar(tmp, ps, s127, MAGIC, mult, add)
                y_t = workA.tile([P, 1024], f16, tag="yA")
                nc.vector.tensor_scalar(y_t, tmp, MAGIC, None, subtract)
                if tt == NT - 1:
                    nc.sync.dma_start(am_loc.rearrange("1 (t p) -> p t", p=P),
                                      am_all)
                    nc.gpsimd.collective_compute(
                        "AllGather", mybir.AluOpType.bypass,
                        replica_groups=GROUPS,
                        ins=[am_loc.opt()], outs=[am_g.opt()])
                for kc in range(DK):
                    pst = psT.tile([P, P], f16, tag="trA")
                    nc.tensor.transpose(pst, y_t[:, kc * P:(kc + 1) * P], ident16)
                    nc.vector.tensor_copy(yT_sb[:, kc, tt * P:(tt + 1) * P], pst)
            sA.__exit__(None, None, None)
            sG = nc.named_scope("gather"); sG.__enter__()
            if NT > 1:
                nc.sync.dma_start(
                    yT_loc0.rearrange("(o p) t -> p o t", p=P),
                    yT_sb[:, :, 0:L // 2])
                nc.gpsimd.collective_compute(
                    "AllGather", mybir.AluOpType.bypass, replica_groups=GROUPS,
                    ins=[yT_loc0.opt()], outs=[yT_g0.opt()])
            else:
                nc.sync.dma_start(
                    yT_loc0.rearrange("(o p) t -> p o t", p=P),
                    yT_sb[:, :, 0:L // 2])
                nc.gpsimd.collective_compute(
                    "AllGather", mybir.AluOpType.bypass, replica_groups=GROUPS,
                    ins=[yT_loc0.opt()], outs=[yT_g0.opt()])
            nc.sync.dma_start(
                yT_loc1.rearrange("(o p) t -> p o t", p=P),
                yT_sb[:, :, L // 2:L])
            nc.gpsimd.collective_compute(
                "AllGather", mybir.AluOpType.bypass, replica_groups=GROUPS,
                ins=[yT_loc1.opt()], outs=[yT_g1.opt()])
            sG.__exit__(None, None, None)

        # ---------------- Phase B: QKV ------------------------------------
        fin = tc.alloc_tile_pool(name="fin", bufs=1)
        attn = tc.alloc_tile_pool(name="attn", bufs=1)
        QsT = attn.tile([P, BT], f16)
        KsT = attn.tile([P, BT], f16)
        V_A = attn.tile([P, VT, 65], f16)
        V_B = attn.tile([P, VT, 65], f16)

        with tc.tile_pool(name="gath", bufs=1) as gath, \
             tc.tile_pool(name="workB", bufs=4) as workB, \
             tc.tile_pool(name="psB", bufs=2, space="PSUM") as psB:
            sB = nc.named_scope("phaseB"); sB.__enter__()
            yTg = gath.tile([P, DK, BT], f16)
            yv0 = yT_g0.rearrange("(a o p) t -> a p o t", p=P, o=DK)
            yv1 = yT_g1.rearrange("(a o p) t -> a p o t", p=P, o=DK)
            for peer in range(N_CORES):
                for half, yv in ((0, yv0), (1, yv1)):
                    stg = workB.tile([P, DK, L // 2], i8, tag="stg")
                    nc.sync.dma_start(stg, yv[peer])
                    base = peer * L + half * (L // 2)
                    nc.vector.tensor_copy(yTg[:, :, base:base + L // 2], stg)

            A_q = gath.tile([P, BT], f32)
            A_k = gath.tile([P, BT], f32)
            nc.sync.dma_start(
                A_k, am_g.rearrange("a l -> (a l)")[None, :].to_broadcast((P, BT)))
            nc.vector.tensor_scalar(A_q, A_k, csb[:, 0:1], None, mult)
            nc.vector.tensor_scalar(A_k, A_k, 1.0 / 127.0, None, mult)
            Av = gath.tile([P, VT], f32)
            nc.sync.dma_start(
                Av, am_g.rearrange("a (t p) -> p (a t)", p=P))
            nc.vector.tensor_scalar(Av, Av, csb[:, 1:2], None, mult)

            wq = gath.tile([P, DK, P], f16)
            nc.sync.dma_start(wq, WqT.ap().rearrange("(o p) m -> p o m", p=P))
            wk = gath.tile([P, DK, P], f16)
            nc.sync.dma_start(wk, WkT.ap().rearrange("(o p) m -> p o m", p=P))
            wv = gath.tile([P, DK, P], f16)
            nc.sync.dma_start(wv, WvT.ap().rearrange("(o p) m -> p o m", p=P))

            TBW = min(512, L // 2)
            for tb in range(BT // TBW):
                sl = slice(tb * TBW, (tb + 1) * TBW)
                psq = psB.tile([P, TBW], f32, tag="psq")
                for kc in range(DK):
                    nc.tensor.matmul(psq, wq[:, kc], yTg[:, kc, sl],
                                     start=(kc == 0), stop=(kc == DK - 1))
                nc.vector.tensor_tensor(QsT[:, sl], psq, A_q[:, sl], mult)
                psk = psB.tile([P, TBW], f32, tag="psk")
                for kc in range(DK):
                    nc.tensor.matmul(psk, wk[:, kc], yTg[:, kc, sl],
                                     start=(kc == 0), stop=(kc == DK - 1))
                nc.vector.tensor_tensor(KsT[:, sl], psk, A_k[:, sl], mult)

            nc.vector.memset(V_A[:, :, 64:65], 1.0)
            nc.vector.memset(V_B[:, :, 64:65], 1.0)
            for vt in range(VT):
                psv = psB.tile([P, P], f32, tag="psv")
                for kc in range(DK):
                    nc.tensor.matmul(psv, yTg[:, kc, vt * P:(vt + 1) * P],
                                     wv[:, kc],
                                     start=(kc == 0), stop=(kc == DK - 1))
                nc.vector.tensor_scalar(V_A[:, vt, 0:64], psv[:, 0:64],
                                        Av[:, vt:vt + 1], None, mult)
                nc.vector.tensor_scalar(V_B[:, vt, 0:64], psv[:, 64:128],
                                        Av[:, vt:vt + 1], None, mult)

        # ---------------- Phase C: attention ------------------------------
        sB.__exit__(None, None, None)
        wo = fin.tile([P, DK, D], f16)
        nc.sync.dma_start(wo, WoT.ap().rearrange("(o p) n -> p o n", p=P))

        with tc.tile_pool(name="workC", bufs=6) as workC, \
             tc.tile_pool(name="dramC", bufs=3, space="DRAM") as dramC, \
             tc.tile_pool(name="psS", bufs=2, space="PSUM") as psS_pool, \
             tc.tile_pool(name="psO", bufs=2, space="PSUM") as psO_pool:
            sC = nc.named_scope("phaseC"); sC.__enter__()
            a2a_in_v = a2a_in.rearrange("(a p) l -> a p l", a=N_CORES)
            for b in range(B):
                for qb in range(NQB):
                    q0 = b * T + qb * QB
                    poA = psO_pool.tile([P, 512], f32, tag="poA")
                    poB = psO_pool.tile([P, 512], f32, tag="poB")
                    for kt in range(NKT):
                        k0 = b * T + kt * P
                        ps = psS_pool.tile([P, 1024], f32, tag="S")
                        nc.tensor.matmul(ps[:, 0:512],
                                         KsT[0:64, k0:k0 + P],
                                         QsT[0:64, q0:q0 + QB],
                                         start=True, stop=True)
                        nc.tensor.matmul(ps[:, 512:1024],
                                         KsT[64:128, k0:k0 + P],
                                         QsT[64:128, q0:q0 + QB],
                                         start=True, stop=True)
                        ex = workC.tile([P, 1024], f16, tag="ex")
                        nc.scalar.activation(ex, ps, Exp)
                        vt = (b * T) // P + kt
                        nc.tensor.matmul(poA[0:65], V_A[:, vt], ex[:, 0:512],
                                         start=(kt == 0), stop=(kt == NKT - 1))
                        nc.tensor.matmul(poB[0:65], V_B[:, vt], ex[:, 512:1024],
                                         start=(kt == 0), stop=(kt == NKT - 1))
                    for head, po in ((0, poA), (1, poB)):
                        rrow = workC.tile([1, QB], f32, tag="rrow")
                        nc.vector.reciprocal(rrow, po[64:65, 0:QB])
                        rdr = dramC.tile([1, QB], f32, tag="rdr")
                        nc.sync.dma_start(rdr, rrow)
                        rbc = workC.tile([64, QB], f32, tag="rbc")
                        nc.sync.dma_start(rbc, rdr.to_broadcast((64, QB)))
                        onrm = workC.tile([64, QB], f32, tag="onrm")
                        nc.vector.tensor_tensor(onrm, po[0:64], rbc, mult)
                        step = min(QB, L)
                        for j in range(QB // step):
                            peer, tl = divmod(q0 + j * step, L)
                            nc.sync.dma_start(
                                a2a_in_v[peer, head * 64:(head + 1) * 64,
                                         tl:tl + step],
                                onrm[:, j * step:(j + 1) * step])

            sC.__exit__(None, None, None)
            sA2A = nc.named_scope("a2a"); sA2A.__enter__()
            nc.gpsimd.collective_compute(
                "AllToAll", mybir.AluOpType.bypass, replica_groups=GROUPS,
                ins=[a2a_in.opt()], outs=[a2a_out.opt()])
            sA2A.__exit__(None, None, None)

        attn.release()

        # ---------------- Phase D: final quant + output projection --------
        with tc.tile_pool(name="workD", bufs=4) as workD, \
             tc.tile_pool(name="finD", bufs=1) as finD, \
             tc.tile_pool(name="psD", bufs=2, space="PSUM") as psD, \
             tc.tile_pool(name="psTD", bufs=2, space="PSUM") as psTD:
            sD = nc.named_scope("phaseD"); sD.__enter__()
            outT = finD.tile([P, DK, L], f32)
            a2a_out_v = a2a_out.rearrange("(a p) l -> a p l", a=N_CORES)
            for peer in range(N_CORES):
                nc.sync.dma_start(outT[:, peer, :], a2a_out_v[peer])
            outf = finD.tile([P, NT, D], f32)
            for tt in range(NT):
                for o in range(DK):
                    pst = psTD.tile([P, P], f32, tag="trD2")
                    nc.tensor.transpose(pst, outT[:, o, tt * P:(tt + 1) * P],
                                        ident32)
                    nc.vector.tensor_copy(outf[:, tt, o * P:(o + 1) * P], pst)
            y2T = finD.tile([P, DK, L], f16)
            a2r = finD.tile([P, NT], f32)
            for tt in range(NT):
                am2 = workD.tile([P, 1], f32, tag="am2")
                nc.vector.reduce_max(am2, outf[:, tt], axis=X,
                                     apply_absolute_value=True)
                nc.vector.tensor_scalar_max(am2, am2, 1e-5)
                nc.vector.tensor_tensor(a2r[:, tt:tt + 1], am2, csb[:, 2:3], mult)
                rec = workD.tile([P, 1], f32, tag="recD")
                nc.vector.reciprocal(rec, am2)
                s127 = workD.tile([P, 1], f32, tag="s127D")
                nc.vector.tensor_scalar_mul(s127, rec, 127.0)
                tmp = workD.tile([P, D], f32, tag="tmpD")
                nc.vector.tensor_scalar(tmp, outf[:, tt], s127, MAGIC, mult, add)
                y2 = workD.tile([P, D], f16, tag="y2")
                nc.vector.tensor_scalar(y2, tmp, MAGIC, None, subtract)
                for kc in range(DK):
                    pst = psTD.tile([P, P], f16, tag="trD")
                    nc.tensor.transpose(pst, y2[:, kc * P:(kc + 1) * P], ident16)
                    nc.vector.tensor_copy(y2T[:, kc, tt * P:(tt + 1) * P], pst)
            for tt in range(NT):
                for nh in range(2):
                    psz = psD.tile([P, 512], f32, tag="psz")
                    for kc in range(DK):
                        nc.tensor.matmul(psz, y2T[:, kc, tt * P:(tt + 1) * P],
                                         wo[:, kc, nh * 512:(nh + 1) * 512],
                                         start=(kc == 0), stop=(kc == DK - 1))
                    zsb = workD.tile([P, 512], f32, tag="zsb")
                    nc.vector.tensor_scalar(zsb, psz, a2r[:, tt:tt + 1], None,
                                            mult)
                    nc.sync.dma_start(
                        z.ap()[tt * P:(tt + 1) * P, nh * 512:(nh + 1) * 512],
                        zsb)

        sD.__exit__(None, None, None)
        fin.release()
        dram.release()
        cpool.release()

    nc.compile()
    return nc


def _get_nc(T):
    if T not in _BUILD_CACHE:
        _BUILD_CACHE[T] = _build(T)
    return _BUILD_CACHE[T]


def _wquant(w):
    # reference: scale = 1/clip(mean|w|,1e-5); u = clip(round(w*scale),-1,1)/scale
    scale = np.float32(1.0) / np.maximum(
        np.float32(np.mean(np.abs(w), dtype=np.float64)), np.float32(1e-5))
    u = np.clip(np.rint(w * scale), -1, 1).astype(np.float32)
    return u, np.float32(1.0) / scale  # ternary, dequant scale (= clipped mean)


def kernel(x, mask, Wq, Wk, Wv, Wo, H):
    from concourse.bass_utils import run_bass_kernel_spmd

    x = np.asarray(x, np.float32)
    Wq = np.asarray(Wq, np.float32); Wk = np.asarray(Wk, np.float32)
    Wv = np.asarray(Wv, np.float32); Wo = np.asarray(Wo, np.float32)
    H = np.asarray(H, np.float32)
    Bx, T, Dx = x.shape
    BT = Bx * T
    L = BT // N_CORES

    nc = _get_nc(T)

    xf = x.reshape(BT, Dx)
    x_hi = xf.astype(np.float16)
    x_lo = (xf - x_hi.astype(np.float32)).astype(np.float16)
    H16 = H.astype(np.float16)

    uq, cq = _wquant(Wq); uk, ck = _wquant(Wk)
    uv, cv = _wquant(Wv); uo, co = _wquant(Wo)
    uqT = np.ascontiguousarray(uq.T.astype(np.float16))
    ukT = np.ascontiguousarray(uk.T.astype(np.float16))
    uvT = np.ascontiguousarray(uv.T.astype(np.float16))
    uoT = np.ascontiguousarray(uo.T.astype(np.float16))

    c0 = np.float32(cq) * np.float32(ck) / (np.float32(np.sqrt(DH)) * np.float32(127.0))
    c1 = np.float32(cv) / np.float32(127.0)
    c2 = np.float32(co) / np.float32(127.0)
    consts = np.array([[c0, c1, c2, 0.0]], np.float32)

    in_maps = []
    for c in range(N_CORES):
        rows = slice(c * L, (c + 1) * L)
        cols = slice(c * P, (c + 1) * P)
        in_maps.append({
            "xT_hi": np.ascontiguousarray(x_hi[rows].T),
            "xT_lo": np.ascontiguousarray(x_lo[rows].T),
            "Hm": H16,
            "WqT": np.ascontiguousarray(uqT[:, cols]),
            "WkT": np.ascontiguousarray(ukT[:, cols]),
            "WvT": np.ascontiguousarray(uvT[:, cols]),
            "WoT": uoT,
            "consts": consts,
        })

    res = run_bass_kernel_spmd(nc, in_maps, core_ids=list(range(N_CORES)))
    kernel.last_results = res
    z = np.concatenate([res.results[c]["z"] for c in range(N_CORES)], axis=0)
    return z.reshape(Bx, T, Dx).astype(np.float32)


# revision 25
# speedup vs baseline: 1.0995x; 1.0357x over previous
"""BitNet attention TRN2 kernel: 8-way SPMD (2 heads/core, tokens sharded 8-way).

Dataflow per core c (tokens Tc = rows [c*L,(c+1)*L) of the flattened [B*T, D]
activations, heads {2c, 2c+1}):
  A) x_had = x @ H  via fp16 hi/lo split matmuls (fp32 accumulate);
     per-token int8 absmax quant -> y (fp16-held small ints); PE-transpose;
     AllGather y.T and the per-token absmax across the 8 cores.
  B) Q/K/V projections in the integer domain (exact in fp16), dequantized with
     per-token scales (DMA-broadcast absmax vector x host weight-scale consts).
  C) Per (batch, head): S.T = Ks.T^T @ Qs.T (row-packed head pairs), exp on ACT,
     out.T = [V | 1]^T @ expS.T accumulated over key tiles (ones column gives the
     softmax denominator), normalize, PE-transpose to token-major, AllToAll so
     each core gets its own tokens x all 1024 columns.
  D) Second absmax quant, z = y2 @ Wo_u.T (integer domain), per-token dequant.

Host side quantizes weights to ternary {-1,0,1} (fp16-exact), splits/transposes
x, and concatenates the 8 z slices.
"""
import sys

if "/opt/trn_rl_repo" not in sys.path:
    sys.path.insert(0, "/opt/trn_rl_repo")

import numpy as np

P = 128
D = 1024
NH = 16
DH = 64
B = 2
N_CORES = 8
MAGIC = 12582912.0  # 1.5 * 2**23: fp32 round-to-nearest-int via add/sub

_BUILD_CACHE = {}


def _build(T):
    import concourse.bass as bass  # noqa: F401
    import concourse.mybir as mybir
    import concourse.tile as tile
    from concourse import bacc
    from concourse.masks import make_identity

    f16 = mybir.dt.float16
    f32 = mybir.dt.float32
    i8 = mybir.dt.int8
    Exp = mybir.ActivationFunctionType.Exp
    mult = mybir.AluOpType.mult
    add = mybir.AluOpType.add
    subtract = mybir.AluOpType.subtract
    X = mybir.AxisListType.X

    BT = B * T
    L = BT // N_CORES          # tokens per core
    NT = L // P                # local token tiles
    DK = D // P                # contraction chunks
    QB = 512                   # query block
    NQB = T // QB              # query blocks per batch
    NKT = T // P               # key tiles per batch
    VT = BT // P               # global token tiles (for V)
    GROUPS = [list(range(N_CORES))]

    nc = bacc.Bacc("TRN2", target_bir_lowering=False, debug=False,
                   num_devices=N_CORES)

    # I/O
    xT_hi = nc.dram_tensor("xT_hi", [D, L], f16, kind="ExternalInput")
    xT_lo = nc.dram_tensor("xT_lo", [D, L], f16, kind="ExternalInput")
    Hm = nc.dram_tensor("Hm", [D, D], f16, kind="ExternalInput")
    WqT = nc.dram_tensor("WqT", [D, P], f16, kind="ExternalInput")
    WkT = nc.dram_tensor("WkT", [D, P], f16, kind="ExternalInput")
    WvT = nc.dram_tensor("WvT", [D, P], f16, kind="ExternalInput")
    WoT = nc.dram_tensor("WoT", [D, D], f16, kind="ExternalInput")
    consts = nc.dram_tensor("consts", [1, 4], f32, kind="ExternalInput")
    z = nc.dram_tensor("z", [L, D], f32, kind="ExternalOutput")

    with tile.TileContext(nc) as tc:
        cpool = tc.alloc_tile_pool(name="cpool", bufs=1)
        dram = tc.alloc_tile_pool(name="dram", bufs=1, space="DRAM")

        ident16 = cpool.tile([P, P], f16)
        make_identity(nc, ident16)
        ident32 = cpool.tile([P, P], f32)
        make_identity(nc, ident32)
        csb = cpool.tile([P, 4], f32)
        nc.sync.dma_start(csb, consts.ap().to_broadcast((P, 4)))

        # DRAM intermediates
        yT_loc0 = dram.tile([D, L // 2], i8)
        yT_loc1 = dram.tile([D, L // 2], i8)
        yT_g0 = dram.tile([N_CORES * D, L // 2], i8, addr_space="Shared")
        yT_g1 = dram.tile([N_CORES * D, L // 2], i8, addr_space="Shared")
        am_loc = dram.tile([1, L], f32)
        am_g = dram.tile([N_CORES, L], f32, addr_space="Shared")
        a2a_in = dram.tile([N_CORES * P, L], f32)
        a2a_out = dram.tile([N_CORES * P, L], f32)
        y2_loc = dram.tile([L, D], f16)

        # ---------------- Phase A: x@H, quant, transpose, gather ----------
        with tc.tile_pool(name="pre", bufs=1) as pre, \
             tc.tile_pool(name="workA", bufs=3) as workA, \
             tc.tile_pool(name="psA", bufs=2, space="PSUM") as psA, \
             tc.tile_pool(name="psT", bufs=4, space="PSUM") as psT:
            sA = nc.named_scope("phaseA"); sA.__enter__()
            xhi = pre.tile([P, DK, L], f16)
            xlo = pre.tile([P, DK, L], f16)
            Hsb = pre.tile([P, DK, D], f16)
            xhi_v = xT_hi.ap().rearrange("(o p) t -> p o t", p=P)
            xlo_v = xT_lo.ap().rearrange("(o p) t -> p o t", p=P)
            H_v = Hm.ap().rearrange("(o p) d -> p o d", p=P)
            for kc in range(DK):
                nc.sync.dma_start(Hsb[:, kc], H_v[:, kc])
                nc.sync.dma_start(xhi[:, kc], xhi_v[:, kc])
                nc.sync.dma_start(xlo[:, kc], xlo_v[:, kc])
            yT_sb = pre.tile([P, DK, L], i8)
            am_all = pre.tile([P, NT], f32)

            for tt in range(NT):
                ps = psA.tile([P, 1024], f32, tag="xh")
                for half in range(2):
                    for kc in range(DK):
                        nc.tensor.matmul(
                            ps[:, half * 512:(half + 1) * 512],
                            xhi[:, kc, tt * P:(tt + 1) * P],
                            Hsb[:, kc, half * 512:(half + 1) * 512],
                            start=(kc == 0), stop=False)
                        nc.tensor.matmul(
                            ps[:, half * 512:(half + 1) * 512],
                            xlo[:, kc, tt * P:(tt + 1) * P],
                            Hsb[:, kc, half * 512:(half + 1) * 512],
                            start=False, stop=(kc == DK - 1))
                am_t = am_all[:, tt:tt + 1]
                nc.vector.reduce_max(am_t, ps, axis=X, apply_absolute_value=True)
                nc.vector.tensor_scalar_max(am_t, am_t, 1e-5)
                rec = workA.tile([P, 1], f32, tag="rec")
                nc.vector.reciprocal(rec, am_t)
                s127 = workA.tile([P, 1], f32, tag="s127")
                nc.vector.tensor_scalar_mul(s127, rec, 127.0)
                tmp = workA.tile([P, 1024], f32, tag="tmpA")
                nc.vector.tensor_scalar(tmp, ps, s127, MAGIC, mult, add)
                y_t = workA.tile([P, 1024], f16, tag="yA")
                nc.vector.tensor_scalar(y_t, tmp, MAGIC, None, subtract)
                if tt == NT - 1:
                    nc.sync.dma_start(am_loc.rearrange("1 (t p) -> p t", p=P),
                                      am_all)
                    nc.gpsimd.collective_compute(
                        "AllGather", mybir.AluOpType.bypass,
                        replica_groups=GROUPS,
                        ins=[am_loc.opt()], outs=[am_g.opt()])
                for kc in range(DK):
                    pst = psT.tile([P, P], f16, tag="trA")
                    nc.tensor.transpose(pst, y_t[:, kc * P:(kc + 1) * P], ident16)
                    nc.vector.tensor_copy(yT_sb[:, kc, tt * P:(tt + 1) * P], pst)
            sA.__exit__(None, None, None)
            sG = nc.named_scope("gather"); sG.__enter__()
            if NT > 1:
                nc.sync.dma_start(
                    yT_loc0.rearrange("(o p) t -> p o t", p=P),
                    yT_sb[:, :, 0:L // 2])
                nc.gpsimd.collective_compute(
                    "AllGather", mybir.AluOpType.bypass, replica_groups=GROUPS,
                    ins=[yT_loc0.opt()], outs=[yT_g0.opt()])
            else:
                nc.sync.dma_start(
                    yT_loc0.rearrange("(o p) t -> p o t", p=P),
                    yT_sb[:, :, 0:L // 2])
                nc.gpsimd.collective_compute(
                    "AllGather", mybir.AluOpType.bypass, replica_groups=GROUPS,
                    ins=[yT_loc0.opt()], outs=[yT_g0.opt()])
            nc.sync.dma_start(
                yT_loc1.rearrange("(o p) t -> p o t", p=P),
                yT_sb[:, :, L // 2:L])
            nc.gpsimd.collective_compute(
                "AllGather", mybir.AluOpType.bypass, replica_groups=GROUPS,
                ins=[yT_loc1.opt()], outs=[yT_g1.opt()])
            sG.__exit__(None, None, None)

        # ---------------- Phase B: QKV ------------------------------------
        fin = tc.alloc_tile_pool(name="fin", bufs=1)
        attn = tc.alloc_tile_pool(name="attn", bufs=1)
        QsT = attn.tile([P, BT], f16)
        KsT = attn.tile([P, BT], f16)
        V_A = attn.tile([P, VT, 65], f16)
        V_B = attn.tile([P, VT, 65], f16)

        with tc.tile_pool(name="gath", bufs=1) as gath, \
             tc.tile_pool(name="workB", bufs=4) as workB, \
             tc.tile_pool(name="psB", bufs=2, space="PSUM") as psB:
            sB = nc.named_scope("phaseB"); sB.__enter__()
            yTg = gath.tile([P, DK, BT], f16)
            yv0 = yT_g0.rearrange("(a o p) t -> a p o t", p=P, o=DK)
            yv1 = yT_g1.rearrange("(a o p) t -> a p o t", p=P, o=DK)
            for peer in range(N_CORES):
                for half, yv in ((0, yv0), (1, yv1)):
                    stg = workB.tile([P, DK, L // 2], i8, tag="stg")
                    nc.sync.dma_start(stg, yv[peer])
                    base = peer * L + half * (L // 2)
                    nc.vector.tensor_copy(yTg[:, :, base:base + L // 2], stg)

            A_q = gath.tile([P, BT], f32)
            A_k = gath.tile([P, BT], f32)
            nc.sync.dma_start(
                A_k, am_g.rearrange("a l -> (a l)")[None, :].to_broadcast((P, BT)))
            nc.vector.tensor_scalar(A_q, A_k, csb[:, 0:1], None, mult)
            nc.vector.tensor_scalar(A_k, A_k, 1.0 / 127.0, None, mult)
            Av = gath.tile([P, VT], f32)
            nc.sync.dma_start(
                Av, am_g.rearrange("a (t p) -> p (a t)", p=P))
            nc.vector.tensor_scalar(Av, Av, csb[:, 1:2], None, mult)

            wq = gath.tile([P, DK, P], f16)
            nc.sync.dma_start(wq, WqT.ap().rearrange("(o p) m -> p o m", p=P))
            wk = gath.tile([P, DK, P], f16)
            nc.sync.dma_start(wk, WkT.ap().rearrange("(o p) m -> p o m", p=P))
            wv = gath.tile([P, DK, P], f16)
            nc.sync.dma_start(wv, WvT.ap().rearrange("(o p) m -> p o m", p=P))

            TBW = min(512, L // 2)
            for tb in range(BT // TBW):
                sl = slice(tb * TBW, (tb + 1) * TBW)
                psq = psB.tile([P, TBW], f32, tag="psq")
                for kc in range(DK):
                    nc.tensor.matmul(psq, wq[:, kc], yTg[:, kc, sl],
                                     start=(kc == 0), stop=(kc == DK - 1))
                nc.vector.tensor_tensor(QsT[:, sl], psq, A_q[:, sl], mult)
                psk = psB.tile([P, TBW], f32, tag="psk")
                for kc in range(DK):
                    nc.tensor.matmul(psk, wk[:, kc], yTg[:, kc, sl],
                                     start=(kc == 0), stop=(kc == DK - 1))
                nc.vector.tensor_tensor(KsT[:, sl], psk, A_k[:, sl], mult)

            nc.vector.memset(V_A[:, :, 64:65], 1.0)
            nc.vector.memset(V_B[:, :, 64:65], 1.0)
            for vt in range(VT):
                psv = psB.tile([P, P], f32, tag="psv")
                for kc in range(DK):
                    nc.tensor.matmul(psv, yTg[:, kc, vt * P:(vt + 1) * P],
                                     wv[:, kc],
                                     start=(kc == 0), stop=(kc == DK - 1))
                nc.vector.tensor_scalar(V_A[:, vt, 0:64], psv[:, 0:64],
                                        Av[:, vt:vt + 1], None, mult)
                nc.vector.tensor_scalar(V_B[:, vt, 0:64], psv[:, 64:128],
                                        Av[:, vt:vt + 1], None, mult)

        # ---------------- Phase C: attention ------------------------------
        sB.__exit__(None, None, None)
        wo = fin.tile([P, DK, D], f16)
        nc.sync.dma_start(wo, WoT.ap().rearrange("(o p) n -> p o n", p=P))

        with tc.tile_pool(name="workC", bufs=6) as workC, \
             tc.tile_pool(name="dramC", bufs=3, space="DRAM") as dramC, \
             tc.tile_pool(name="psS", bufs=2, space="PSUM") as psS_pool, \
             tc.tile_pool(name="psO", bufs=2, space="PSUM") as psO_pool:
            sC = nc.named_scope("phaseC"); sC.__enter__()
            a2a_in_v = a2a_in.rearrange("(a p) l -> a p l", a=N_CORES)
            for b in range(B):
                for qb in range(NQB):
                    q0 = b * T + qb * QB
                    poA = psO_pool.tile([P, 512], f32, tag="poA")
                    poB = psO_pool.tile([P, 512], f32, tag="poB")
                    for kt in range(NKT):
                        k0 = b * T + kt * P
                        ps = psS_pool.tile([P, 1024], f32, tag="S")
                        nc.tensor.matmul(ps[:, 0:512],
                                         KsT[0:64, k0:k0 + P],
                                         QsT[0:64, q0:q0 + QB],
                                         start=True, stop=True)
                        nc.tensor.matmul(ps[:, 512:1024],
                                         KsT[64:128, k0:k0 + P],
                                         QsT[64:128, q0:q0 + QB],
                                         start=True, stop=True)
                        ex = workC.tile([P, 1024], f16, tag="ex")
                        nc.scalar.activation(ex, ps, Exp)
                        vt = (b * T) // P + kt
                        nc.tensor.matmul(poA[0:65], V_A[:, vt], ex[:, 0:512],
                                         start=(kt == 0), stop=(kt == NKT - 1))
                        nc.tensor.matmul(poB[0:65], V_B[:, vt], ex[:, 512:1024],
                                         start=(kt == 0), stop=(kt == NKT - 1))
                    for head, po in ((0, poA), (1, poB)):
                        rrow = workC.tile([1, QB], f32, tag="rrow")
                        nc.vector.reciprocal(rrow, po[64:65, 0:QB])
                        rdr = dramC.tile([1, QB], f32, tag="rdr")
                        nc.sync.dma_start(rdr, rrow)
                        rbc = workC.tile([64, QB], f32, tag="rbc")
                        nc.sync.dma_start(rbc, rdr.to_broadcast((64, QB)))
                        onrm = workC.tile([64, QB], f32, tag="onrm")
                        nc.vector.tensor_tensor(onrm, po[0:64], rbc, mult)
                        step = min(QB, L)
                        for j in range(QB // step):
                            peer, tl = divmod(q0 + j * step, L)
                            nc.sync.dma_start(
                                a2a_in_v[peer, head * 64:(head + 1) * 64,
                                         tl:tl + step],
                                onrm[:, j * step:(j + 1) * step])

            sC.__exit__(None, None, None)
            sA2A = nc.named_scope("a2a"); sA2A.__enter__()
            nc.gpsimd.collective_compute(
                "AllToAll", mybir.AluOpType.bypass, replica_groups=GROUPS,
                ins=[a2a_in.opt()], outs=[a2a_out.opt()])
            sA2A.__exit__(None, None, None)

        attn.release()

        # ---------------- Phase D: final quant + output projection --------
        with tc.tile_pool(name="workD", bufs=4) as workD, \
             tc.tile_pool(name="finD", bufs=1) as finD, \
             tc.tile_pool(name="psD", bufs=2, space="PSUM") as psD, \
             tc.tile_pool(name="psTD", bufs=2, space="PSUM") as psTD:
            sD = nc.named_scope("phaseD"); sD.__enter__()
            outT = finD.tile([P, DK, L], f32)
            a2a_out_v = a2a_out.rearrange("(a p) l -> a p l", a=N_CORES)
            for peer in range(N_CORES):
                nc.sync.dma_start(outT[:, peer, :], a2a_out_v[peer])
            outf = finD.tile([P, NT, D], f32)
            for tt in range(NT):
                for o in range(DK):
                    pst = psTD.tile([P, P], f32, tag="trD2")
                    nc.tensor.transpose(pst, outT[:, o, tt * P:(tt + 1) * P],
                                        ident32)
                    nc.vector.tensor_copy(outf[:, tt, o * P:(o + 1) * P], pst)
            y2T = finD.tile([P, DK, L], f16)
            a2r = finD.tile([P, NT], f32)
            for tt in range(NT):
                am2 = workD.tile([P, 1], f32, tag="am2")
                nc.vector.reduce_max(am2, outf[:, tt], axis=X,
                                     apply_absolute_value=True)
                nc.vector.tensor_scalar_max(am2, am2, 1e-5)
                nc.vector.tensor_tensor(a2r[:, tt:tt + 1], am2, csb[:, 2:3], mult)
                rec = workD.tile([P, 1], f32, tag="recD")
                nc.vector.reciprocal(rec, am2)
                s127 = workD.tile([P, 1], f32, tag="s127D")
                nc.vector.tensor_scalar_mul(s127, rec, 127.0)
                tmp = workD.tile([P, D], f32, tag="tmpD")
                nc.vector.tensor_scalar(tmp, outf[:, tt], s127, MAGIC, mult, add)
                y2 = workD.tile([P, D], f16, tag="y2")
                nc.vector.tensor_scalar(y2, tmp, MAGIC, None, subtract)
                for kc in range(DK):
                    pst = psTD.tile([P, P], f16, tag="trD")
                    nc.tensor.transpose(pst, y2[:, kc * P:(kc + 1) * P], ident16)
                    nc.vector.tensor_copy(y2T[:, kc, tt * P:(tt + 1) * P], pst)
            for tt in range(NT):
                for nh in range(2):
                    psz = psD.tile([P, 512], f32, tag="psz")
                    for kc in range(DK):
                        nc.tensor.matmul(psz, y2T[:, kc, tt * P:(tt + 1) * P],
                                         wo[:, kc, nh * 512:(nh + 1) * 512],
                                         start=(kc == 0), stop=(kc == DK - 1))
                    zsb = workD.tile([P, 512], f32, tag="zsb")
                    nc.vector.tensor_scalar(zsb, psz, a2r[:, tt:tt + 1], None,
                                            mult)
                    nc.sync.dma_start(
                        z.ap()[tt * P:(tt + 1) * P, nh * 512:(nh + 1) * 512],
                        zsb)

        sD.__exit__(None, None, None)
        fin.release()
        dram.release()
        cpool.release()

    nc.compile()
    return nc


def _get_nc(T):
    if T not in _BUILD_CACHE:
        _BUILD_CACHE[T] = _build(T)
    return _BUILD_CACHE[T]


def _wquant(w):
    # reference: scale = 1/clip(mean|w|,1e-5); u = clip(round(w*scale),-1,1)/scale
    scale = np.float32(1.0) / np.maximum(
        np.float32(np.mean(np.abs(w), dtype=np.float64)), np.float32(1e-5))
    u = np.clip(np.rint(w * scale), -1, 1).astype(np.float32)
    return u, np.float32(1.0) / scale  # ternary, dequant scale (= clipped mean)


def kernel(x, mask, Wq, Wk, Wv, Wo, H):
    from concourse.bass_utils import run_bass_kernel_spmd

    x = np.asarray(x, np.float32)
    Wq = np.asarray(Wq, np.float32); Wk = np.asarray(Wk, np.float32)
    Wv = np.asarray(Wv, np.float32); Wo = np.asarray(Wo, np.float32)
    H = np.asarray(H, np.float32)
    Bx, T, Dx = x.shape
    BT = Bx * T
    L = BT // N_CORES

    nc = _get_nc(T)

    xf = x.reshape(BT, Dx)
    x_hi = xf.astype(np.float16)
    x_lo = (xf - x_hi.astype(np.float32)).astype(np.float16)
    H16 = H.astype(np.float16)

    uq, cq = _wquant(Wq); uk, ck = _wquant(Wk)
    uv, cv = _wquant(Wv); uo, co = _wquant(Wo)
    uqT = np.ascontiguousarray(uq.T.astype(np.float16))
    ukT = np.ascontiguousarray(uk.T.astype(np.float16))
    uvT = np.ascontiguousarray(uv.T.astype(np.float16))
    uoT = np.ascontiguousarray(uo.T.astype(np.float16))

    c0 = np.float32(cq) * np.float32(ck) / (np.float32(np.sqrt(DH)) * np.float32(127.0))
    c1 = np.float32(cv) / np.float32(127.0)
    c2 = np.float32(co) / np.float32(127.0)
    consts = np.array([[c0, c1, c2, 0.0]], np.float32)

    in_maps = []
    for c in range(N_CORES):
        rows = slice(c * L, (c + 1) * L)
        cols = slice(c * P, (c + 1) * P)
        in_maps.append({
            "xT_hi": np.ascontiguousarray(x_hi[rows].T),
            "xT_lo": np.ascontiguousarray(x_lo[rows].T),
            "Hm": H16,
            "WqT": np.ascontiguousarray(uqT[:, cols]),
            "WkT": np.ascontiguousarray(ukT[:, cols]),
            "WvT": np.ascontiguousarray(uvT[:, cols]),
            "WoT": uoT,
            "consts": consts,
        })

    res = run_bass_kernel_spmd(nc, in_maps, core_ids=list(range(N_CORES)))
    kernel.last_results = res
    z = np.concatenate([res.results[c]["z"] for c in range(N_CORES)], axis=0)
    return z.reshape(Bx, T, Dx).astype(np.float32)
